# revision 18
# baseline (speedup 1.0000x reference)
"""AthenaSA sliding-window attention layer on 8 TRN2 NeuronCores.

Sharding: sequence-parallel. 8 cores = 2 batches x 4 sequence chunks of 1024
tokens. Each core recomputes k/v for a 512-token halo (zero-padded for the
first chunk), so there are NO collectives — the kernel is embarrassingly
parallel and each core runs an identical Bass graph on different data.

Per-core pipeline (projections fp8-e4m3 with DoubleRow perf mode, all matmuls
at N=512 moving columns so the PE streams at 1 col/cycle with LDWEIGHTS fully
hidden; attention bf16 scores, accumulation f32):
  emb8 [128, 16, 1536] fp8 (host-pre-packed k-tile-pair layout) -> RMSNorm
  stats via fp8 ones-matmul partition reduction -> QK projections in
  transposed layout (q^T, k^T = [dk, tokens]) via fp8 DoubleRow + RoPE
  (partition-shifted reads) -> V projection in natural layout (fp8 DR) ->
  banded sliding-window attention processed per kv-head QUAD (the 4 GQA query
  heads of one kv head share scores/probs tiles at 512 columns), per-chunk
  softmax exp straight to fp8, triangular window masks applied on DVE, the
  first-block zero-halo correction folded into a per-core dn-weights tensor
  (halo V rows are zero so only the softmax denominator needs correcting) ->
  fp8 DoubleRow out-projection back to natural layout + residual.
"""
import math
import os
import sys

sys.path.insert(0, "/opt/trn_rl_repo")

import numpy as np
import ml_dtypes

import concourse.bass as bass
import concourse.bacc as bacc
import concourse.mybir as mybir
from concourse import tile
from concourse import bass_utils
from contextlib import ExitStack

BF16 = ml_dtypes.bfloat16
E4M3 = ml_dtypes.float8_e4m3

B, S, E = 2, 4096, 2048
H, HKV, DK, DV = 16, 4, 128, 128
WIN = 512
EPS = 1e-5
TOWN, TALL, HALO = 1024, 1536, 512
NE = E // 128            # 16 e-tiles
NP = NE // 2             # 8 e-tile pairs (DoubleRow)
NB = 2                   # window blocks per core
NQC = 4                  # query tiles of 128 per block
NCH = 5                  # key chunks of 128 per 640-window
GQ = H // HKV            # 4 query heads per kv head (one "quad")
NWARM = 26               # HAM warm-up dummy matmuls issued at t=0

f32 = mybir.dt.float32
bf = mybir.dt.bfloat16
f8 = mybir.dt.float8e4
AF = mybir.ActivationFunctionType
DR = mybir.MatmulPerfMode.DoubleRow
ALU = mybir.AluOpType

# fp8 weights are stored pre-scaled by WSCALE (power of 2) to sit in
# e4m3's normal range (raw values have sigma ~1/sqrt(E) = 0.022, partly
# denormal in e4m3). Compensated downstream: q/k via host cos/sin buffers,
# v via the rts copy scale, out-projection via the residual-add scale.
WSCALE = 32.0
# fp8 probs scale (applied via the exp bias): keeps exp(score) under e4m3
# max (240) while small probs stay above the denormal flush. Cancels
# exactly in the softmax ratio (numerator and denominator both carry it).
PSCALE = 1.0 / 8.0


def build(tc, d):
    nc = tc.nc

    with ExitStack() as stage_all:
        stage_all.enter_context(
            nc.allow_low_precision(reason="fp8/bf16 compute path by design"))
        const_pool = stage_all.enter_context(tc.tile_pool(name="const", bufs=1))
        ones = const_pool.tile([128, 128], bf)
        nc.gpsimd.memset(ones[:], 1.0)
        warmmv = const_pool.tile([128, 512], bf)
        nc.gpsimd.memset(warmmv[:], 0.0)
        epsb = const_pool.tile([128, 1], f32)
        nc.gpsimd.memset(epsb[:], EPS)
        epsw = const_pool.tile([128, 1], f32)
        nc.gpsimd.memset(epsw[:], EPS * WSCALE * WSCALE)
        # exp bias: probs = exp(score + ln(PSCALE)) = exp(score)*PSCALE
        lnps = const_pool.tile([128, 1], f32)
        nc.gpsimd.memset(lnps[:], math.log(PSCALE))

        # HAM warm-up: the PE clock-gate defaults to 4/8 (1.2 GHz) and only
        # releases after ~3.4us of sustained matmul activity. The first real
        # matmul can't start until the first 1MB emb8 chunk lands (~8-10us),
        # which would leave the whole first compute phase half-rate. Spin
        # dummy matmuls on memset tiles to pre-warm during the DMA window.
        with tc.tile_pool(name="warm_ps", bufs=1, space="PSUM") as warm_ps:
            wps = warm_ps.tile([128, 512], f32)
            for _ in range(NWARM):
                nc.tensor.matmul(wps[:], ones[:], warmmv[:],
                                 start=True, stop=True)

        # manually-scoped pools; LIFO open/close order
        acat_cm = tc.tile_pool(name="acat", bufs=HKV)          # ..D
        acat_pool = acat_cm.__enter__()
        wo_cm = tc.tile_pool(name="wo", bufs=1)               # ..D
        wo_pool = wo_cm.__enter__()
        emb_cm = tc.tile_pool(name="embown", bufs=1)          # ..D
        emb_pool = emb_cm.__enter__()
        msk_cm = tc.tile_pool(name="msk", bufs=1)             # ..D
        msk_pool = msk_cm.__enter__()
        kT_cm = tc.tile_pool(name="kT", bufs=HKV)             # ..C
        kT_pool = kT_cm.__enter__()
        v_cm = tc.tile_pool(name="v", bufs=1)                 # ..C
        v_pool = v_cm.__enter__()
        qT_cm = tc.tile_pool(name="qT", bufs=HKV)             # ..C
        qT_pool = qT_cm.__enter__()
        emb8_cm = tc.tile_pool(name="emb8", bufs=1)           # ..B2
        emb8_pool = emb8_cm.__enter__()
        wkv_cm = tc.tile_pool(name="wkv", bufs=1)             # ..B1
        wkv_pool = wkv_cm.__enter__()
        wq_cm = tc.tile_pool(name="wqp", bufs=6)              # ..B1
        wq_pool = wq_cm.__enter__()
        rb = const_pool.tile([128, TALL], bf)                 # 1/rms, all rows
        rts = const_pool.tile([128, 12], f32)                 # 1/rms per token-tile

        # emb8: whole residual-stream chunk in fp8, token-chunked
        # k-tile-pair layout [128, 3(chunk of 512 tok), NE, 512].
        # Split DMAs so several queues pull concurrently.
        # DMA order matters: chunk c0 feeds the first Gram/V matmuls — fan it
        # across every DMA queue; wv8 is needed ~2us in, before c1/c2.
        emb8 = emb8_pool.tile([128, 3, NE, 512], f8)
        for e in range(NE):
            nc.sync.dma_start(emb8[:, 0, e, :], d["emb8"][:, 0, e, :])
        ident = const_pool.tile([128, 128], bf)
        nc.sync.dma_start(ident[:], d["ident"][:])
        # chunk c1 before wv8: the c1 Gram matmuls come up before the first
        # V matmul needs wv8
        for half in range(8):
            nc.sync.dma_start(emb8[:, 1, 2 * half:2 * (half + 1), :],
                              d["emb8"][:, 1, 2 * half:2 * (half + 1), :])
        wv8 = wkv_pool.tile([128, NE, HKV * DV], f8)
        for qtr in range(4):
            nc.sync.dma_start(wv8[:, 4 * qtr:4 * (qtr + 1), :],
                              d["wv8"][:, 4 * qtr:4 * (qtr + 1), :])
        for qtr in range(4):
            nc.sync.dma_start(emb8[:, 2, 4 * qtr:4 * (qtr + 1), :],
                              d["emb8"][:, 2, 4 * qtr:4 * (qtr + 1), :])
        wk8 = wkv_pool.tile([128, NE, HKV * DK], f8)
        nc.sync.dma_start(wk8[:], d["wk8"][:])
        # tiny attention-mask / dn-weight tensors: land long before stage C
        tri = msk_pool.tile([128, 2, GQ * 128], f8)
        nc.sync.dma_start(tri[:], d["tri"][:])
        dnw = msk_pool.tile([128, 12, 128], f8)
        nc.sync.dma_start(dnw[:], d["dnw"][:])

        def load_wqh(h):
            """per-head wq tile, ring of 6: issue the DMA only after the
            previous occupant's matmuls are on record (WAR safety)."""
            wqh = wq_pool.tile([128, NE, DK], f8, name="wqh")
            for hf in range(2):
                nc.sync.dma_start(wqh[:, 8 * hf:8 * (hf + 1), :],
                                  d["wq8"][h][:, 8 * hf:8 * (hf + 1), :])
            return wqh

        def embsl(pe, off, w):
            """emb8 [128, 2(e pair), w] AP at global token offset off."""
            c, o = divmod(off, 512)
            assert o + w <= 512
            return emb8[:, c, 2 * pe:2 * pe + 2, o:o + w]

        # ---------------- Stage B: V, K^T, Q^T projections ----------------
        kT = []   # per kv head: [128(dk), TALL] bf16, rope'd
        qTq = []  # per kv head: [128(dk), 8 tiles x 4 heads x 128 q] bf16
        with ExitStack() as sb1:
            gi_pool = sb1.enter_context(tc.tile_pool(name="gi", bufs=3))
            r_pool = sb1.enter_context(tc.tile_pool(name="rms", bufs=1))
            cs_pool = sb1.enter_context(tc.tile_pool(name="cosk", bufs=1))
            tmp_pool = sb1.enter_context(tc.tile_pool(name="ropetmp", bufs=1))
            tmpq_pool = sb1.enter_context(tc.tile_pool(name="ropetmpq", bufs=2))

            cosk = cs_pool.tile([128, TALL], bf)
            sink = cs_pool.tile([128, TALL], bf)
            nc.sync.dma_start(cosk[:], d["coskT"][:, :])
            nc.sync.dma_start(sink[:], d["sinkT"][:, :])
            cosq = cs_pool.tile([128, TOWN], bf)
            sinq = cs_pool.tile([128, TOWN], bf)
            nc.sync.dma_start(cosq[:], d["cosqT"][:, :])
            nc.sync.dma_start(sinq[:], d["sinqT"][:, :])
            # deep wq prefetch: first 6 heads in flight before stage A ends
            wqh_tiles = [load_wqh(h) for h in range(6)]

            # ---- RMSNorm stats + V projection, pipelined per 512-token
            # chunk so PE work tracks the arriving emb8 DMAs. ssq per token
            # via PE Gram diagonals: G_t = emb_t^T emb_t (fp8 DR), GI_t =
            # G_t * I (DVE); rts (v-scale) via free-reduce(GI_t) and a
            # per-chunk sqrt/recip so V drains without waiting on rb.
            rts_raw = r_pool.tile([128, 12], f32)
            s_rt = r_pool.tile([128, 12], f32)
            gis = []
            v_all = v_pool.tile([128, 12, HKV * DV], f8)
            with ExitStack() as sa_ps:
                g_psum = sa_ps.enter_context(
                    tc.tile_pool(name="g_ps", bufs=2, space="PSUM"))
                ssq_psum = sa_ps.enter_context(
                    tc.tile_pool(name="ssq_ps", bufs=1, space="PSUM"))
                vps_pool = sa_ps.enter_context(
                    tc.tile_pool(name="v_ps", bufs=3, space="PSUM"))
                ssq = ssq_psum.tile([128, TALL], f32)  # 3 banks

                for c in range(3):
                    for t in range(4 * c, 4 * c + 4):
                        # pad G tiles to a full PSUM bank so accumulation
                        # groups of different t never share a bank
                        # (interleaved-start hazard)
                        g = g_psum.tile([128, 512], f32)
                        for pe in range(NP):
                            nc.tensor.matmul(
                                g[:, 0:128], embsl(pe, t * 128, 128),
                                embsl(pe, t * 128, 128),
                                start=(pe == 0), stop=(pe == NP - 1),
                                perf_mode=DR)
                        gi = gi_pool.tile([128, 128], bf)
                        nc.vector.tensor_mul(gi[:], g[:, 0:128], ident[:])
                        nc.vector.tensor_reduce(
                            rts_raw[:, t:t + 1], gi[:],
                            axis=mybir.AxisListType.X, op=ALU.add)
                        gis.append(gi)
                    # rts = 1/sqrt(ssq/E+eps)/WSCALE for this chunk's tiles:
                    # sqrt(WSCALE^2*(ssq/E + eps)) then plain reciprocal
                    csl = slice(4 * c, 4 * c + 4)
                    nc.scalar.activation(s_rt[:, csl], rts_raw[:, csl],
                                         AF.Sqrt, bias=epsw[:],
                                         scale=WSCALE * WSCALE / E)
                    nc.vector.reciprocal_approx_fast(rts[:, csl], s_rt[:, csl])
                    # V for this chunk's 4 token tiles; all 12 live in ONE
                    # fp8 tile so attention can take [128, 2(key-tile), 128]
                    # DoubleRow slices across tile pairs.
                    for t in range(4 * c, 4 * c + 4):
                        vps = vps_pool.tile([128, HKV * DV], f32)  # 1 bank
                        for pe in range(NP):
                            nc.tensor.matmul(
                                vps[:],
                                embsl(pe, t * 128, 128),
                                wv8[:, 2 * pe:2 * pe + 2, :],
                                start=(pe == 0), stop=(pe == NP - 1),
                                perf_mode=DR)
                        nc.vector.tensor_scalar_mul(v_all[:, t, :], vps[:],
                                                    rts[:, t:t + 1])

                # rb = 1/sqrt(ssq/E + eps), all rows identical, via
                # ssq row-broadcast = ones^T @ GI_t (single-instruction
                # groups into ssq regions are sequential-safe). Only the
                # k/q rope factors consume rb.
                for t in range(12):
                    nc.tensor.matmul(ssq[:, t * 128:(t + 1) * 128], ones[:],
                                     gis[t][:], start=True, stop=True)
                s_sb = r_pool.tile([128, TALL], f32)
                nc.scalar.activation(s_sb[:], ssq[:], AF.Sqrt,
                                     bias=epsb[:], scale=1.0 / E)
                nc.vector.reciprocal_approx_fast(s_sb[:], s_sb[:])
                nc.vector.tensor_copy(rb[:], s_sb[:])        # cast -> bf16
                # pre-load the Exp activation table while the PE is busy:
                # the first real exp otherwise pays a ~1.3us ACT_TABLE_LOAD
                # right at the stage-B/C transition.
                nc.scalar.activation(s_rt[:, 0:1], rts_raw[:, 0:1], AF.Exp,
                                     bias=lnps[:], scale=-1.0)

            nc.vector.tensor_mul(cosk[:], cosk[:], rb[:])
            nc.vector.tensor_mul(sink[:], sink[:], rb[:])
            nc.vector.tensor_mul(cosq[:], cosq[:], rb[:, HALO:])
            nc.vector.tensor_mul(sinq[:], sinq[:], rb[:, HALO:])

            with tc.tile_pool(name="q_ps", bufs=4, space="PSUM") as qps_pool:
                kps_cm = tc.tile_pool(name="k_ps", bufs=4, space="PSUM")
                kps_pool = kps_cm.__enter__()
                for hk in range(HKV):
                    # rope: ko = cos*kraw + sin*swap(kraw), emitted per
                    # 512-token chunk right behind the chunk's matmul chain
                    # so the drain tail after the last matmul stays short
                    # (the stage-C psum pools can't open until every rope
                    # read of this pool completes).
                    ksw = tmp_pool.tile([128, TALL], bf)
                    t1 = tmp_pool.tile([128, TALL], bf)
                    ko = kT_pool.tile([128, TALL], bf, name="ko")
                    for s3 in range(3):
                        kps = kps_pool.tile([128, 512], f32, name="kps")
                        for pe in range(NP):
                            nc.tensor.matmul(
                                kps[:],
                                wk8[:, 2 * pe:2 * pe + 2,
                                    hk * DK:(hk + 1) * DK],
                                embsl(pe, s3 * 512, 512),
                                start=(pe == 0), stop=(pe == NP - 1),
                                perf_mode=DR)
                        sl = slice(s3 * 512, (s3 + 1) * 512)
                        nc.scalar.copy(ksw[0:64, sl], kps[64:128, :])
                        nc.scalar.copy(ksw[64:128, sl], kps[0:64, :])
                        nc.vector.tensor_mul(t1[:, sl], kps[:], cosk[:, sl])
                        nc.vector.tensor_mul(ko[:, sl], ksw[:, sl],
                                             sink[:, sl])
                        nc.vector.tensor_add(ko[:, sl], ko[:, sl], t1[:, sl])
                    kT.append(ko)
                kps_cm.__exit__(None, None, None)

                # ---------------- Q^T projection ----------------
                # wq comes host-permuted per-head [H, 128, NE, DK] so a
                # head's weights DMA contiguously; 6-deep prefetch ring.
                for h in range(H):
                    kv, g = divmod(h, GQ)
                    if g == 0:
                        qquad = qT_pool.tile([128, 4 * TOWN], bf, name="qquad")
                        qTq.append(qquad)
                    wqh = wqh_tiles[h]
                    qsw = tmpq_pool.tile([128, TOWN], bf)
                    t1 = tmpq_pool.tile([128, TOWN], bf, name="t1q")
                    qo = qquad.rearrange(
                        "p (t g q) -> p t g q", g=GQ, q=128)[:, :, g, :]
                    for s2 in range(2):
                        qps = qps_pool.tile([128, 512], f32, name="qps")
                        for pe in range(NP):
                            nc.tensor.matmul(
                                qps[:],
                                wqh[:, 2 * pe:2 * pe + 2, :],
                                embsl(pe, HALO + s2 * 512, 512),
                                start=(pe == 0), stop=(pe == NP - 1),
                                perf_mode=DR)
                        # per-chunk rope (swap copies must ride on ACT: the
                        # cross-partition GpSimd copy crashes walrus)
                        sl = slice(s2 * 512, (s2 + 1) * 512)
                        nc.scalar.copy(qsw[0:64, sl], qps[64:128, :])
                        nc.scalar.copy(qsw[64:128, sl], qps[0:64, :])
                        nc.vector.tensor_mul(t1[:, sl], qps[:], cosq[:, sl])
                        qo2 = qo[:, 4 * s2:4 * (s2 + 1), :]
                        nc.vector.tensor_mul(qo2, qsw[:, sl], sinq[:, sl])
                        nc.vector.tensor_add(qo2, qo2, t1[:, sl])
                    if h + 6 < H:
                        wqh_tiles.append(load_wqh(h + 6))
        wq_cm.__exit__(None, None, None)
        wkv_cm.__exit__(None, None, None)
        emb8_cm.__exit__(None, None, None)

        # ---------------- Stage C: attention ----------------
        # One iteration = one (q-tile, kv-head quad): the 4 GQA query heads
        # of a kv head share the 512-column scores/probs tiles.
        acatq = []
        for kv in range(HKV):
            acatq.append(acat_pool.tile([128, 8 * GQ * 128], f8, name="acatq"))

        # out-projection weights land j-major so the first out-projection
        # only waits on its own 1MB slice; residual tiles stream in a ring
        wo8 = wo_pool.tile([128, 4, H, 512], f8)
        for j in range(4):
            nc.sync.dma_start(wo8[:, j, :, :], d["wo8"][:, j, :, :])
        emb_own = emb_pool.tile([128, 3, E], bf)
        for t in range(3):
            nc.sync.dma_start(emb_own[:, t, :],
                              d["emb_own"][t * 128:(t + 1) * 128, :])

        with ExitStack() as sc_stage:
            probs_pool = sc_stage.enter_context(tc.tile_pool(name="probs", bufs=3))
            rec_pool = sc_stage.enter_context(tc.tile_pool(name="rec", bufs=4))
            out_pool = sc_stage.enter_context(tc.tile_pool(name="outsb", bufs=2))
            scps_pool = sc_stage.enter_context(
                tc.tile_pool(name="sc_ps", bufs=1, space="PSUM"))
            red_pool = sc_stage.enter_context(
                tc.tile_pool(name="red_ps", bufs=3, space="PSUM"))

            def emit_reduce(probs, blk, qc, kv):
                """dn/rec/attention-out/acat for one (tile, kv quad)."""
                t = 4 * blk + qc
                pr5 = probs[:]  # [128, 5, 512]
                # denominator first so its reciprocal (DVE) overlaps the
                # attention-out matmuls. dn stationary is the per-core
                # validity-weights tensor: 0-columns exclude the zero-halo
                # keys of the first sequence chunk from the denominator
                # (their V rows are zero, so the numerator needs no mask).
                dn = red_pool.tile([128, 512], f32, name="red")
                for ch in (0, 2):
                    nc.tensor.matmul(dn[:], dnw[:, t + ch:t + ch + 2, :],
                                     pr5[:, ch:ch + 2, :],
                                     start=(ch == 0), stop=False,
                                     perf_mode=DR)
                nc.tensor.matmul(dn[:], dnw[:, t + 4, :], pr5[:, 4, :],
                                 start=False, stop=True)
                rec = rec_pool.tile([128, 512], f32)
                nc.vector.reciprocal_approx_fast(rec[:], dn[:])
                otp = red_pool.tile([128, 512], f32, name="red")
                for ch in (0, 2):
                    nc.tensor.matmul(
                        otp[:],
                        v_all[:, t + ch:t + ch + 2, kv * DV:(kv + 1) * DV],
                        pr5[:, ch:ch + 2, :],
                        start=(ch == 0), stop=False, perf_mode=DR)
                nc.tensor.matmul(
                    otp[:], v_all[:, t + 4, kv * DV:(kv + 1) * DV],
                    pr5[:, 4, :], start=False, stop=True)
                nc.vector.tensor_mul(acatq[kv][:, t * 512:(t + 1) * 512],
                                     otp[:], rec[:])

            outsb = {}

            def emit_outproj_unit(t, j):
                """One 512-column chunk of the out projection + residual for
                q-tile t. Units are spread one-per-iteration through the
                attention stream so the PE always has independent fill work
                while an iteration's exp/mask chain drains. The accumulator
                shares the red psum ring (same tag)."""
                if j == 0:
                    if 1 <= t <= 5:
                        # slot free after tile t-1's adds; stream tile t+2
                        nc.sync.dma_start(
                            emb_own[:, (t + 2) % 3, :],
                            d["emb_own"][(t + 2) * 128:(t + 3) * 128, :])
                    outsb[t] = out_pool.tile([128, E], bf, name="out_sb")
                out_sb = outsb[t]
                op = red_pool.tile([128, 512], f32, name="red")
                for kv in range(HKV):
                    for h2 in range(2):
                        pidx = 4 * kv + 2 * h2
                        lhs = acatq[kv].rearrange(
                            "p (t g q) -> p t g q", g=GQ, q=128)[
                                :, t, 2 * h2:2 * h2 + 2, :]
                        nc.tensor.matmul(
                            op[:], lhs, wo8[:, j, pidx:pidx + 2, :],
                            start=(kv == 0 and h2 == 0),
                            stop=(kv == HKV - 1 and h2 == 1),
                            perf_mode=DR)
                nc.vector.scalar_tensor_tensor(
                    out_sb[:, j * 512:(j + 1) * 512],
                    op[:], 1.0 / WSCALE,
                    emb_own[:, t % 3, j * 512:(j + 1) * 512],
                    ALU.mult, ALU.add)
                # per-slice output DMA overlaps the remaining matmuls
                nc.sync.dma_start(
                    d["out"][t * 128:(t + 1) * 128, j * 512:(j + 1) * 512],
                    out_sb[:, j * 512:(j + 1) * 512])
                if j == 3:
                    del outsb[t]

            # Software-pipelined by one (tile, quad) step: the PE queue
            # alternates scores_i / reduce_{i-1} / one out-proj unit, so the
            # reduce matmuls never sit behind a wait on their own
            # iteration's exp+mask chain.
            pending = None
            opq = []
            for blk in range(NB):
                for qc in range(NQC):
                    t = 4 * blk + qc              # own q-tile index
                    for kv in range(HKV):
                        scp = scps_pool.tile([128, NCH, 512], f32)  # 5 banks
                        probs = probs_pool.tile([128, NCH, 512], f8)
                        for ch in range(NCH):
                            nc.tensor.matmul(
                                scp[:, ch, :],
                                kT[kv][:, (t + ch) * 128:(t + ch + 1) * 128],
                                qTq[kv][:, t * 512:(t + 1) * 512],
                                start=True, stop=True)
                            if ch == 1:
                                # exp of the first two chunks right behind
                                # their score matmuls: chunk-0 probs (and its
                                # mask) are ready well before the reduce
                                # needs them, and the chunk-0/1 psum banks
                                # free early for the next iteration
                                nc.scalar.activation(probs[:, 0:2, :],
                                                     scp[:, 0:2, :], AF.Exp,
                                                     bias=lnps[:], scale=1.0)
                        nc.scalar.activation(probs[:, 2:5, :], scp[:, 2:5, :],
                                             AF.Exp, bias=lnps[:], scale=1.0)
                        # triangular window masks on the boundary chunks,
                        # split across DVE and GpSimd so they run in
                        # parallel. (Uniform across tiles and cores; the
                        # zero-halo case is handled by dnw + zero V rows
                        # instead of masks.)
                        nc.vector.tensor_mul(probs[:, 0, :], probs[:, 0, :],
                                             tri[:, 0, :])
                        nc.gpsimd.tensor_mul(probs[:, 4, :], probs[:, 4, :],
                                             tri[:, 1, :])
                        if pending is not None:
                            emit_reduce(*pending)
                            if pending[3] == HKV - 1:
                                opq += [(4 * pending[1] + pending[2], j)
                                        for j in range(4)]
                        if opq:
                            emit_outproj_unit(*opq.pop(0))
                            if len(opq) > 4:
                                emit_outproj_unit(*opq.pop(0))
                        pending = (probs, blk, qc, kv)
            emit_reduce(*pending)
            opq += [(4 * pending[1] + pending[2], j) for j in range(4)]
            for unit in opq:
                emit_outproj_unit(*unit)
        qT_cm.__exit__(None, None, None)
        v_cm.__exit__(None, None, None)
        kT_cm.__exit__(None, None, None)

        msk_cm.__exit__(None, None, None)
        emb_cm.__exit__(None, None, None)
        wo_cm.__exit__(None, None, None)
        acat_cm.__exit__(None, None, None)


_CACHED_NC = None


def build_graph():
    global _CACHED_NC
    if _CACHED_NC is not None:
        return _CACHED_NC
    nc = bacc.Bacc("TRN2", target_bir_lowering=False, debug=False,
                   enable_asserts=False, num_devices=8)
    d = {}
    d["emb8"] = nc.dram_tensor("emb8", [128, 3, NE, 512], f8,
                               kind="ExternalInput").ap()
    d["ident"] = nc.dram_tensor("ident", [128, 128], bf,
                                kind="ExternalInput").ap()
    d["emb_own"] = nc.dram_tensor("emb_own", [TOWN, E], bf,
                                  kind="ExternalInput").ap()
    d["wq8"] = nc.dram_tensor("wq8", [H, 128, NE, DK], f8,
                              kind="ExternalInput").ap()
    d["wk8"] = nc.dram_tensor("wk8", [128, NE, HKV * DK], f8,
                              kind="ExternalInput").ap()
    d["wv8"] = nc.dram_tensor("wv8", [128, NE, HKV * DV], f8,
                              kind="ExternalInput").ap()
    d["wo8"] = nc.dram_tensor("wo8", [128, 4, H, 512], f8,
                              kind="ExternalInput").ap()
    d["cosqT"] = nc.dram_tensor("cosqT", [DK, TOWN], bf, kind="ExternalInput").ap()
    d["sinqT"] = nc.dram_tensor("sinqT", [DK, TOWN], bf, kind="ExternalInput").ap()
    d["coskT"] = nc.dram_tensor("coskT", [DK, TALL], bf, kind="ExternalInput").ap()
    d["sinkT"] = nc.dram_tensor("sinkT", [DK, TALL], bf, kind="ExternalInput").ap()
    d["tri"] = nc.dram_tensor("tri", [128, 2, GQ * 128], f8,
                              kind="ExternalInput").ap()
    d["dnw"] = nc.dram_tensor("dnw", [128, 12, 128], f8,
                              kind="ExternalInput").ap()
    d["out"] = nc.dram_tensor("out", [TOWN, E], bf, kind="ExternalOutput").ap()

    with tile.TileContext(nc, trace_sim=False) as tc:
        build(tc, d)
    nc.compile()
    _CACHED_NC = nc
    return nc


def make_in_maps(embeddings, cos_buffer, sin_buffer, wq, wk, wv, wo):
    embeddings = np.asarray(embeddings, dtype=np.float32)
    cos_buffer = np.asarray(cos_buffer, dtype=np.float32)
    sin_buffer = np.asarray(sin_buffer, dtype=np.float32)
    # [E, H*DK] -> [H, 128, NE, DK] fp8 (k-tile-pair packed, per head).
    # Weights pre-scaled by WSCALE for e4m3 range; the whole 1/sqrt(DK)
    # score scale plus both WSCALE compensations ride on the q-side
    # cos/sin (q) and k-side cos/sin (k) host buffers.
    ws = float(WSCALE)
    wq_s = np.asarray(wq, np.float32) * ws
    wq_s = wq_s.reshape(NE, 128, H, DK).transpose(2, 1, 0, 3)
    wq8 = np.ascontiguousarray(wq_s).astype(E4M3)
    # [E, HKV*DK] -> [128, NE, HKV*DK]
    wk8 = np.ascontiguousarray(
        (np.asarray(wk, np.float32) * ws).reshape(NE, 128, HKV * DK)
        .transpose(1, 0, 2)).astype(E4M3)
    wv8 = np.ascontiguousarray(
        (np.asarray(wv, np.float32) * ws).reshape(NE, 128, HKV * DV)
        .transpose(1, 0, 2)).astype(E4M3)
    # [H*DV, E] -> [128(dv), 4(j), H, 512] (j-major output column chunks)
    wo8 = np.ascontiguousarray(
        (np.asarray(wo, np.float32) * ws).reshape(H, DV, 4, 512)
        .transpose(1, 2, 0, 3)).astype(E4M3)
    aq = 1.0 / (ws * math.sqrt(DK))   # q-side compensation (+ score scale)
    ak = 1.0 / ws                     # k-side compensation

    # triangular window masks, replicated per quad head: [128(j), 2, 4(g),
    # 128(i)] -> keep j>i for the oldest chunk, j<=i for the newest.
    jj = np.arange(128)[:, None]
    ii = np.arange(128)[None, :]
    tri = np.zeros((128, 2, GQ, 128), np.float32)
    tri[:, 0, :, :] = (jj > ii)[:, None, :]
    tri[:, 1, :, :] = (jj <= ii)[:, None, :]
    tri = tri.reshape(128, 2, GQ * 128).astype(E4M3)

    in_maps = []
    for core in range(8):
        b, c = divmod(core, 4)
        tok0 = 1024 * c
        if c == 0:
            pad = np.zeros((HALO, E), np.float32)
            seg = np.concatenate([pad, embeddings[b, :TOWN]], axis=0)
            padc = np.zeros((HALO, DK), np.float32)
            ck = np.concatenate([padc, cos_buffer[1, 0, :TOWN]], axis=0)
            sk = np.concatenate([padc, sin_buffer[1, 0, :TOWN]], axis=0)
        else:
            seg = embeddings[b, tok0 - HALO:tok0 + TOWN]
            ck = cos_buffer[1, 0, tok0 - HALO:tok0 + TOWN]
            sk = sin_buffer[1, 0, tok0 - HALO:tok0 + TOWN]

        # [TALL, E] -> [128, 3(tok chunk), NE, 512] fp8
        emb8 = np.ascontiguousarray(
            seg.T.reshape(NE, 128, 3, 512).transpose(1, 2, 0, 3)).astype(E4M3)

        # dn validity weights [128(k), 12(key tile), 128(m)]: zero for the
        # zero-padded halo tiles of the first sequence chunk so those keys
        # drop out of the softmax denominator; ones everywhere else.
        dnw = np.ones((128, 12, 128), np.float32)
        if c == 0:
            dnw[:, 0:4, :] = 0.0
        dnw = dnw.astype(E4M3)

        in_maps.append({
            "emb8": emb8,
            "ident": np.eye(128, dtype=np.float32).astype(BF16),
            "emb_own": np.ascontiguousarray(
                embeddings[b, tok0:tok0 + TOWN]).astype(BF16),
            "wq8": wq8, "wk8": wk8, "wv8": wv8, "wo8": wo8,
            "cosqT": np.ascontiguousarray(
                cos_buffer[0, 0, tok0:tok0 + TOWN].T * aq).astype(BF16),
            "sinqT": np.ascontiguousarray(
                sin_buffer[0, 0, tok0:tok0 + TOWN].T * aq).astype(BF16),
            "coskT": np.ascontiguousarray(ck.T * ak).astype(BF16),
            "sinkT": np.ascontiguousarray(sk.T * ak).astype(BF16),
            "tri": tri,
            "dnw": dnw,
        })
    return in_maps


def _install_ntff_hook():
    """Recreate the missing antenv.axon_hooks registry so
    run_bass_kernel_spmd(trace=True) can capture an NTFF profile."""
    import types
    if "antenv.axon_hooks" not in sys.modules:
        m = types.ModuleType("antenv.axon_hooks")
        m._hook = None
        m.set_axon_ntff_profile_hook = lambda h: setattr(m, "_hook", h)
        m.get_axon_ntff_profile_hook = lambda: m._hook
        sys.modules["antenv.axon_hooks"] = m
        try:
            import antenv
            antenv.axon_hooks = m
        except ImportError:
            pass
    try:
        from trn_agent_boot.trn_boot import _ntff_profile_via_ctypes
        hook = _ntff_profile_via_ctypes("/opt/axon/libaxon_pjrt.so")
        sys.modules["antenv.axon_hooks"].set_axon_ntff_profile_hook(hook)
    except Exception as exc:  # degrade to no tracing
        print(f"ntff hook install failed: {exc}", file=sys.stderr)


def kernel(embeddings, cos_buffer, sin_buffer, wq, wk, wv, wo, window_size,
           trace=False):
    assert int(window_size) == WIN
    if trace:
        _install_ntff_hook()
    nc = build_graph()
    in_maps = make_in_maps(embeddings, cos_buffer, sin_buffer, wq, wk, wv, wo)
    if trace:
        # warm-up executions: ramp device clocks so the traced run below
        # measures the steady-state rate
        for _ in range(2):
            bass_utils.run_bass_kernel_spmd(
                nc, in_maps, core_ids=list(range(8)), trace=False)
    res = bass_utils.run_bass_kernel_spmd(
        nc, in_maps, core_ids=list(range(8)), trace=trace)
    out = np.zeros((B, S, E), np.float32)
    for core in range(8):
        b, c = divmod(core, 4)
        out[b, 1024 * c:1024 * (c + 1)] = np.asarray(
            res.results[core]["out"]).astype(np.float32)
    if trace:
        kernel.last_exec_time_ns = res.exec_time_ns
    return out


kernel.last_exec_time_ns = None


# revision 38
# speedup vs baseline: 1.0832x; 1.0832x over previous
"""AthenaSA sliding-window attention layer on 8 TRN2 NeuronCores.

Sharding: sequence-parallel. 8 cores = 2 batches x 4 sequence chunks of 1024
tokens. Each core recomputes k/v for a 512-token halo (zero-padded for the
first chunk), so there are NO collectives — the kernel is embarrassingly
parallel and each core runs an identical Bass graph on different data.

Per-core pipeline (projections fp8-e4m3 with DoubleRow perf mode, all matmuls
at N=512 moving columns so the PE streams at 1 col/cycle with LDWEIGHTS fully
hidden; attention bf16 scores, accumulation f32):
  emb8 [128, 16, 1536] fp8 (host-pre-packed k-tile-pair layout) -> RMSNorm
  stats via fp8 ones-matmul partition reduction -> QK projections in
  transposed layout (q^T, k^T = [dk, tokens]) via fp8 DoubleRow + RoPE
  (partition-shifted reads) -> V projection in natural layout (fp8 DR) ->
  banded sliding-window attention processed per kv-head QUAD (the 4 GQA query
  heads of one kv head share scores/probs tiles at 512 columns), per-chunk
  softmax exp straight to fp8, triangular window masks applied on DVE, the
  first-block zero-halo correction folded into a per-core dn-weights tensor
  (halo V rows are zero so only the softmax denominator needs correcting) ->
  fp8 DoubleRow out-projection back to natural layout + residual.
"""
import math
import os
import sys

sys.path.insert(0, "/opt/trn_rl_repo")

import numpy as np
import ml_dtypes

import concourse.bass as bass
import concourse.bacc as bacc
import concourse.mybir as mybir
from concourse import tile
from concourse import bass_utils
from contextlib import ExitStack

BF16 = ml_dtypes.bfloat16
E4M3 = ml_dtypes.float8_e4m3

B, S, E = 2, 4096, 2048
H, HKV, DK, DV = 16, 4, 128, 128
WIN = 512
EPS = 1e-5
TOWN, TALL, HALO = 1024, 1536, 512
NE = E // 128            # 16 e-tiles
NP = NE // 2             # 8 e-tile pairs (DoubleRow)
NB = 2                   # window blocks per core
NQC = 4                  # query tiles of 128 per block
NCH = 5                  # key chunks of 128 per 640-window
GQ = H // HKV            # 4 query heads per kv head (one "quad")
NWARM = 26               # HAM warm-up dummy matmuls issued at t=0

f32 = mybir.dt.float32
bf = mybir.dt.bfloat16
f8 = mybir.dt.float8e4
AF = mybir.ActivationFunctionType
DR = mybir.MatmulPerfMode.DoubleRow
ALU = mybir.AluOpType

# fp8 weights are stored pre-scaled by WSCALE (power of 2) to sit in
# e4m3's normal range (raw values have sigma ~1/sqrt(E) = 0.022, partly
# denormal in e4m3). Compensated downstream: q/k via host cos/sin buffers,
# v via the rts copy scale, out-projection via the residual-add scale.
WSCALE = 32.0
# fp8 probs scale (applied via the exp bias): keeps exp(score) under e4m3
# max (240) while small probs stay above the denormal flush. Cancels
# exactly in the softmax ratio (numerator and denominator both carry it).
PSCALE = 1.0 / 8.0


def build(tc, d):
    nc = tc.nc

    with ExitStack() as stage_all:
        stage_all.enter_context(
            nc.allow_low_precision(reason="fp8/bf16 compute path by design"))
        const_pool = stage_all.enter_context(tc.tile_pool(name="const", bufs=1))
        ones = const_pool.tile([128, 128], bf)
        nc.gpsimd.memset(ones[:], 1.0)
        warmmv = const_pool.tile([128, 512], bf)
        nc.gpsimd.memset(warmmv[:], 0.0)
        epsb = const_pool.tile([128, 1], f32)
        nc.gpsimd.memset(epsb[:], EPS)
        epsw = const_pool.tile([128, 1], f32)
        nc.gpsimd.memset(epsw[:], EPS * WSCALE * WSCALE)
        # exp bias: probs = exp(score + ln(PSCALE)) = exp(score)*PSCALE
        lnps = const_pool.tile([128, 1], f32)
        nc.gpsimd.memset(lnps[:], math.log(PSCALE))

        # HAM warm-up: the PE clock-gate defaults to 4/8 (1.2 GHz) and only
        # releases after ~3.4us of sustained matmul activity. The first real
        # matmul can't start until the first 1MB emb8 chunk lands (~8-10us),
        # which would leave the whole first compute phase half-rate. Spin
        # dummy matmuls on memset tiles to pre-warm during the DMA window.
        with tc.tile_pool(name="warm_ps", bufs=1, space="PSUM") as warm_ps:
            wps = warm_ps.tile([128, 512], f32)
            for _ in range(NWARM):
                nc.tensor.matmul(wps[:], ones[:], warmmv[:],
                                 start=True, stop=True)

        # manually-scoped pools; LIFO open/close order
        acat_cm = tc.tile_pool(name="acat", bufs=HKV)          # ..D
        acat_pool = acat_cm.__enter__()
        wo_cm = tc.tile_pool(name="wo", bufs=1)               # ..D
        wo_pool = wo_cm.__enter__()
        emb_cm = tc.tile_pool(name="embown", bufs=1)          # ..D
        emb_pool = emb_cm.__enter__()
        msk_cm = tc.tile_pool(name="msk", bufs=1)             # ..D
        msk_pool = msk_cm.__enter__()
        # probs opened OUTSIDE the stage-B scope: its SBUF zone must not
        # reuse stage-B pool space, or iteration 0's exp inherits a release
        # dependency on the last Q-head's rope drain. (rec/outsb are first
        # touched several iterations in, when stage B has long drained.)
        probs_cm = tc.tile_pool(name="probs", bufs=3)         # ..D
        probs_pool = probs_cm.__enter__()
        kT_cm = tc.tile_pool(name="kT", bufs=HKV)             # ..C
        kT_pool = kT_cm.__enter__()
        v_cm = tc.tile_pool(name="v", bufs=1)                 # ..C
        v_pool = v_cm.__enter__()
        qT_cm = tc.tile_pool(name="qT", bufs=HKV)             # ..C
        qT_pool = qT_cm.__enter__()
        emb8_cm = tc.tile_pool(name="emb8", bufs=1)           # ..B2
        emb8_pool = emb8_cm.__enter__()
        wkv_cm = tc.tile_pool(name="wkv", bufs=1)             # ..B1
        wkv_pool = wkv_cm.__enter__()
        wq_cm = tc.tile_pool(name="wqp", bufs=6)              # ..B1
        wq_pool = wq_cm.__enter__()
        rb = const_pool.tile([128, TALL], bf)                 # 1/rms, all rows
        rts = const_pool.tile([128, 12], f32)                 # 1/rms per token-tile

        # emb8: whole residual-stream chunk in fp8, token-chunked
        # k-tile-pair layout [128, 3(chunk of 512 tok), NE, 512].
        # Split DMAs so several queues pull concurrently.
        # DMA order matters: chunk c0 feeds the first Gram/V matmuls — fan it
        # across every DMA queue; wv8 is needed ~2us in, before c1/c2.
        emb8 = emb8_pool.tile([128, 3, NE, 512], f8)
        for e in range(NE):
            nc.sync.dma_start(emb8[:, 0, e, :], d["emb8"][:, 0, e, :])
        ident = const_pool.tile([128, 128], bf)
        nc.sync.dma_start(ident[:], d["ident"][:])
        # chunk c1 before wv8: the c1 Gram matmuls come up before the first
        # V matmul needs wv8
        for half in range(8):
            nc.sync.dma_start(emb8[:, 1, 2 * half:2 * (half + 1), :],
                              d["emb8"][:, 1, 2 * half:2 * (half + 1), :])
        wv8 = wkv_pool.tile([128, NE, HKV * DV], f8)
        for qtr in range(4):
            nc.sync.dma_start(wv8[:, 4 * qtr:4 * (qtr + 1), :],
                              d["wv8"][:, 4 * qtr:4 * (qtr + 1), :])
        for qtr in range(4):
            nc.sync.dma_start(emb8[:, 2, 4 * qtr:4 * (qtr + 1), :],
                              d["emb8"][:, 2, 4 * qtr:4 * (qtr + 1), :])
        wk8 = wkv_pool.tile([128, NE, HKV * DK], f8)
        nc.sync.dma_start(wk8[:], d["wk8"][:])
        # tiny attention-mask / dn-weight tensors: land long before stage C
        tri = msk_pool.tile([128, 2, GQ * 128], f8)
        nc.sync.dma_start(tri[:], d["tri"][:])
        dnw = msk_pool.tile([128, 12, 128], f8)
        nc.sync.dma_start(dnw[:], d["dnw"][:])

        def load_wqh(h):
            """per-head wq tile, ring of 6: issue the DMA only after the
            previous occupant's matmuls are on record (WAR safety)."""
            wqh = wq_pool.tile([128, NE, DK], f8, name="wqh")
            for hf in range(2):
                nc.sync.dma_start(wqh[:, 8 * hf:8 * (hf + 1), :],
                                  d["wq8"][h][:, 8 * hf:8 * (hf + 1), :])
            return wqh

        def embsl(pe, off, w):
            """emb8 [128, 2(e pair), w] AP at global token offset off."""
            c, o = divmod(off, 512)
            assert o + w <= 512
            return emb8[:, c, 2 * pe:2 * pe + 2, o:o + w]

        # ---------------- Stage B: V, K^T, Q^T projections ----------------
        kT = []   # per kv head: [128(dk), TALL] bf16, rope'd
        qTq = []  # per kv head: [128(dk), 8 tiles x 4 heads x 128 q] bf16
        with ExitStack() as sb1:
            gi_pool = sb1.enter_context(tc.tile_pool(name="gi", bufs=3))
            r_pool = sb1.enter_context(tc.tile_pool(name="rms", bufs=1))
            cs_pool = sb1.enter_context(tc.tile_pool(name="cosk", bufs=1))
            tmp_pool = sb1.enter_context(tc.tile_pool(name="ropetmp", bufs=1))
            tmpq_pool = sb1.enter_context(tc.tile_pool(name="ropetmpq", bufs=2))

            cosk = cs_pool.tile([128, TALL], bf)
            sink = cs_pool.tile([128, TALL], bf)
            nc.sync.dma_start(cosk[:], d["coskT"][:, :])
            nc.sync.dma_start(sink[:], d["sinkT"][:, :])
            cosq = cs_pool.tile([128, TOWN], bf)
            sinq = cs_pool.tile([128, TOWN], bf)
            nc.sync.dma_start(cosq[:], d["cosqT"][:, :])
            nc.sync.dma_start(sinq[:], d["sinqT"][:, :])
            # deep wq prefetch: first 6 heads in flight before stage A ends
            wqh_tiles = [load_wqh(h) for h in range(6)]

            # ---- RMSNorm stats + V projection, pipelined per 512-token
            # chunk so PE work tracks the arriving emb8 DMAs. ssq per token
            # via PE Gram diagonals: G_t = emb_t^T emb_t (fp8 DR), GI_t =
            # G_t * I (DVE); rts (v-scale) via free-reduce(GI_t) and a
            # per-chunk sqrt/recip so V drains without waiting on rb.
            rts_raw = r_pool.tile([128, 12], f32)
            s_rt = r_pool.tile([128, 12], f32)
            gis = []
            v_all = v_pool.tile([128, 12, HKV * DV], f8)
            with ExitStack() as sa_ps:
                g_psum = sa_ps.enter_context(
                    tc.tile_pool(name="g_ps", bufs=2, space="PSUM"))
                ssq_psum = sa_ps.enter_context(
                    tc.tile_pool(name="ssq_ps", bufs=1, space="PSUM"))
                vps_pool = sa_ps.enter_context(
                    tc.tile_pool(name="v_ps", bufs=3, space="PSUM"))
                ssq = ssq_psum.tile([128, TALL], f32)  # 3 banks

                for c in range(3):
                    for t in range(4 * c, 4 * c + 4):
                        # pad G tiles to a full PSUM bank so accumulation
                        # groups of different t never share a bank
                        # (interleaved-start hazard)
                        g = g_psum.tile([128, 512], f32)
                        for pe in range(NP):
                            nc.tensor.matmul(
                                g[:, 0:128], embsl(pe, t * 128, 128),
                                embsl(pe, t * 128, 128),
                                start=(pe == 0), stop=(pe == NP - 1),
                                perf_mode=DR)
                        gi = gi_pool.tile([128, 128], bf)
                        nc.vector.tensor_mul(gi[:], g[:, 0:128], ident[:])
                        nc.vector.tensor_reduce(
                            rts_raw[:, t:t + 1], gi[:],
                            axis=mybir.AxisListType.X, op=ALU.add)
                        gis.append(gi)
                    # rts = 1/sqrt(ssq/E+eps)/WSCALE for this chunk's tiles:
                    # sqrt(WSCALE^2*(ssq/E + eps)) then plain reciprocal
                    csl = slice(4 * c, 4 * c + 4)
                    nc.scalar.activation(s_rt[:, csl], rts_raw[:, csl],
                                         AF.Sqrt, bias=epsw[:],
                                         scale=WSCALE * WSCALE / E)
                    nc.vector.reciprocal_approx_fast(rts[:, csl], s_rt[:, csl])
                    # V for this chunk's 4 token tiles; all 12 live in ONE
                    # fp8 tile so attention can take [128, 2(key-tile), 128]
                    # DoubleRow slices across tile pairs.
                    for t in range(4 * c, 4 * c + 4):
                        vps = vps_pool.tile([128, HKV * DV], f32)  # 1 bank
                        for pe in range(NP):
                            nc.tensor.matmul(
                                vps[:],
                                embsl(pe, t * 128, 128),
                                wv8[:, 2 * pe:2 * pe + 2, :],
                                start=(pe == 0), stop=(pe == NP - 1),
                                perf_mode=DR)
                        nc.vector.tensor_scalar_mul(v_all[:, t, :], vps[:],
                                                    rts[:, t:t + 1])

                # rb = 1/sqrt(ssq/E + eps), all rows identical, via
                # ssq row-broadcast = ones^T @ GI_t (single-instruction
                # groups into ssq regions are sequential-safe). Only the
                # k/q rope factors consume rb.
                for t in range(12):
                    nc.tensor.matmul(ssq[:, t * 128:(t + 1) * 128], ones[:],
                                     gis[t][:], start=True, stop=True)
                s_sb = r_pool.tile([128, TALL], f32)
                nc.scalar.activation(s_sb[:], ssq[:], AF.Sqrt,
                                     bias=epsb[:], scale=1.0 / E)
                nc.vector.reciprocal_approx_fast(s_sb[:], s_sb[:])
                nc.vector.tensor_copy(rb[:], s_sb[:])        # cast -> bf16

            nc.vector.tensor_mul(cosk[:], cosk[:], rb[:])
            nc.vector.tensor_mul(sink[:], sink[:], rb[:])
            nc.vector.tensor_mul(cosq[:], cosq[:], rb[:, HALO:])
            nc.vector.tensor_mul(sinq[:], sinq[:], rb[:, HALO:])

            with tc.tile_pool(name="q_ps", bufs=4, space="PSUM") as qps_pool:
                kps_cm = tc.tile_pool(name="k_ps", bufs=4, space="PSUM")
                kps_pool = kps_cm.__enter__()
                for hk in range(HKV):
                    # rope: ko = cos*kraw + sin*swap(kraw), emitted per
                    # 512-token chunk right behind the chunk's matmul chain
                    # so the drain tail after the last matmul stays short
                    # (the stage-C psum pools can't open until every rope
                    # read of this pool completes).
                    ksw = tmp_pool.tile([128, TALL], bf)
                    t1 = tmp_pool.tile([128, TALL], bf)
                    ko = kT_pool.tile([128, TALL], bf, name="ko")
                    for s3 in range(3):
                        kps = kps_pool.tile([128, 512], f32, name="kps")
                        for pe in range(NP):
                            nc.tensor.matmul(
                                kps[:],
                                wk8[:, 2 * pe:2 * pe + 2,
                                    hk * DK:(hk + 1) * DK],
                                embsl(pe, s3 * 512, 512),
                                start=(pe == 0), stop=(pe == NP - 1),
                                perf_mode=DR)
                        sl = slice(s3 * 512, (s3 + 1) * 512)
                        nc.scalar.copy(ksw[0:64, sl], kps[64:128, :])
                        nc.scalar.copy(ksw[64:128, sl], kps[0:64, :])
                        nc.vector.tensor_mul(t1[:, sl], kps[:], cosk[:, sl])
                        nc.vector.tensor_mul(ko[:, sl], ksw[:, sl],
                                             sink[:, sl])
                        nc.vector.tensor_add(ko[:, sl], ko[:, sl], t1[:, sl])
                    kT.append(ko)
                kps_cm.__exit__(None, None, None)

                # ---------------- Q^T projection ----------------
                # wq comes host-permuted per-head [H, 128, NE, DK] so a
                # head's weights DMA contiguously; 6-deep prefetch ring.
                for h in range(H):
                    kv, g = divmod(h, GQ)
                    if g == 0:
                        qquad = qT_pool.tile([128, 4 * TOWN], bf, name="qquad")
                        qTq.append(qquad)
                    wqh = wqh_tiles[h]
                    qsw = tmpq_pool.tile([128, TOWN], bf)
                    t1 = tmpq_pool.tile([128, TOWN], bf, name="t1q")
                    qo = qquad.rearrange(
                        "p (t g q) -> p t g q", g=GQ, q=128)[:, :, g, :]
                    for s2 in range(2):
                        qps = qps_pool.tile([128, 512], f32, name="qps")
                        for pe in range(NP):
                            nc.tensor.matmul(
                                qps[:],
                                wqh[:, 2 * pe:2 * pe + 2, :],
                                embsl(pe, HALO + s2 * 512, 512),
                                start=(pe == 0), stop=(pe == NP - 1),
                                perf_mode=DR)
                        # per-chunk rope (swap copies must ride on ACT: the
                        # cross-partition GpSimd copy crashes walrus)
                        sl = slice(s2 * 512, (s2 + 1) * 512)
                        nc.scalar.copy(qsw[0:64, sl], qps[64:128, :])
                        nc.scalar.copy(qsw[64:128, sl], qps[0:64, :])
                        nc.vector.tensor_mul(t1[:, sl], qps[:], cosq[:, sl])
                        qo2 = qo[:, 4 * s2:4 * (s2 + 1), :]
                        nc.vector.tensor_mul(qo2, qsw[:, sl], sinq[:, sl])
                        nc.vector.tensor_add(qo2, qo2, t1[:, sl])
                    if h + 6 < H:
                        wqh_tiles.append(load_wqh(h + 6))
        wq_cm.__exit__(None, None, None)
        wkv_cm.__exit__(None, None, None)
        emb8_cm.__exit__(None, None, None)

        # ---------------- Stage C: attention ----------------
        # One iteration = one (q-tile, kv-head quad): the 4 GQA query heads
        # of a kv head share the 512-column scores/probs tiles.
        acatq = []
        for kv in range(HKV):
            acatq.append(acat_pool.tile([128, 8 * GQ * 128], f8, name="acatq"))

        # out-projection weights land j-major so the first out-projection
        # only waits on its own 1MB slice; residual tiles stream in a ring
        wo8 = wo_pool.tile([128, 4, H, 512], f8)
        for j in range(4):
            nc.sync.dma_start(wo8[:, j, :, :], d["wo8"][:, j, :, :])
        emb_own = emb_pool.tile([128, 3, E], bf)
        for t in range(3):
            nc.sync.dma_start(emb_own[:, t, :],
                              d["emb_own"][t * 128:(t + 1) * 128, :])

        with ExitStack() as sc_stage:
            rec_pool = sc_stage.enter_context(tc.tile_pool(name="rec", bufs=4))
            out_pool = sc_stage.enter_context(tc.tile_pool(name="outsb", bufs=2))
            scps_pool = sc_stage.enter_context(
                tc.tile_pool(name="sc_ps", bufs=1, space="PSUM"))
            red_pool = sc_stage.enter_context(
                tc.tile_pool(name="red_ps", bufs=3, space="PSUM"))

            def emit_reduce(probs, blk, qc, kv):
                """dn/rec/attention-out/acat for one (tile, kv quad)."""
                t = 4 * blk + qc
                pr5 = probs[:]  # [128, 5, 512]
                # denominator first so its reciprocal (DVE) overlaps the
                # attention-out matmuls. dn stationary is the per-core
                # validity-weights tensor: 0-columns exclude the zero-halo
                # keys of the first sequence chunk from the denominator
                # (their V rows are zero, so the numerator needs no mask).
                # chunk order (1,2),(3,4),0: the first DR pair reads only
                # maskless mid-window chunks, so the chain starts on nothing
                # but the exp; the c4 mask gates MM #2 and the c0 mask only
                # the final matmul, hiding the (serial GpSimd) mask latency
                dn = red_pool.tile([128, 512], f32, name="red")
                for ch in (1, 3):
                    nc.tensor.matmul(dn[:], dnw[:, t + ch:t + ch + 2, :],
                                     pr5[:, ch:ch + 2, :],
                                     start=(ch == 1), stop=False,
                                     perf_mode=DR)
                nc.tensor.matmul(dn[:], dnw[:, t, :], pr5[:, 0, :],
                                 start=False, stop=True)
                rec = rec_pool.tile([128, 512], f32)
                nc.vector.reciprocal_approx_fast(rec[:], dn[:])
                otp = red_pool.tile([128, 512], f32, name="red")
                for ch in (1, 3):
                    nc.tensor.matmul(
                        otp[:],
                        v_all[:, t + ch:t + ch + 2, kv * DV:(kv + 1) * DV],
                        pr5[:, ch:ch + 2, :],
                        start=(ch == 1), stop=False, perf_mode=DR)
                nc.tensor.matmul(
                    otp[:], v_all[:, t, kv * DV:(kv + 1) * DV],
                    pr5[:, 0, :], start=False, stop=True)
                nc.vector.tensor_mul(acatq[kv][:, t * 512:(t + 1) * 512],
                                     otp[:], rec[:])

            outsb = {}

            def emit_outproj_unit(t, j):
                """One 512-column chunk of the out projection + residual for
                q-tile t. Units are spread one-per-iteration through the
                attention stream so the PE always has independent fill work
                while an iteration's exp/mask chain drains. The accumulator
                shares the red psum ring (same tag)."""
                if j == 0:
                    if 1 <= t <= 5:
                        # slot free after tile t-1's adds; stream tile t+2
                        nc.sync.dma_start(
                            emb_own[:, (t + 2) % 3, :],
                            d["emb_own"][(t + 2) * 128:(t + 3) * 128, :])
                    outsb[t] = out_pool.tile([128, E], bf, name="out_sb")
                out_sb = outsb[t]
                op = red_pool.tile([128, 512], f32, name="red")
                for kv in range(HKV):
                    for h2 in range(2):
                        pidx = 4 * kv + 2 * h2
                        lhs = acatq[kv].rearrange(
                            "p (t g q) -> p t g q", g=GQ, q=128)[
                                :, t, 2 * h2:2 * h2 + 2, :]
                        nc.tensor.matmul(
                            op[:], lhs, wo8[:, j, pidx:pidx + 2, :],
                            start=(kv == 0 and h2 == 0),
                            stop=(kv == HKV - 1 and h2 == 1),
                            perf_mode=DR)
                nc.vector.scalar_tensor_tensor(
                    out_sb[:, j * 512:(j + 1) * 512],
                    op[:], 1.0 / WSCALE,
                    emb_own[:, t % 3, j * 512:(j + 1) * 512],
                    ALU.mult, ALU.add)
                # per-slice output DMA overlaps the remaining matmuls
                nc.sync.dma_start(
                    d["out"][t * 128:(t + 1) * 128, j * 512:(j + 1) * 512],
                    out_sb[:, j * 512:(j + 1) * 512])
                if j == 3:
                    del outsb[t]

            # Software-pipelined by one (tile, quad) step: the PE queue
            # alternates scores_i / reduce_{i-1} / one out-proj unit, so the
            # reduce matmuls never sit behind a wait on their own
            # iteration's exp+mask chain.
            pending = None
            opq = []
            for blk in range(NB):
                for qc in range(NQC):
                    t = 4 * blk + qc              # own q-tile index
                    for kv in range(HKV):
                        scp = scps_pool.tile([128, NCH, 512], f32)  # 5 banks
                        probs = probs_pool.tile([128, NCH, 512], f8)
                        for ch in range(NCH):
                            nc.tensor.matmul(
                                scp[:, ch, :],
                                kT[kv][:, (t + ch) * 128:(t + ch + 1) * 128],
                                qTq[kv][:, t * 512:(t + 1) * 512],
                                start=True, stop=True)
                        # one whole-tile exp -> fp8. Must stay a single pass
                        # AFTER all five score matmuls: a partial exp read
                        # of the psum tile blocks every later score matmul
                        # into it (psum WAR tracking is tile-granular), which
                        # serializes scores against the scalar engine.
                        nc.scalar.activation(probs[:], scp[:], AF.Exp,
                                             bias=lnps[:], scale=1.0)
                        # triangular window masks on the boundary chunks, on
                        # GpSimd (its own engine: on DVE they head-of-line
                        # block rec/acat). c4 first — the reduce chains are
                        # ordered to need it before c0. (Uniform across
                        # tiles and cores; the zero-halo case is handled by
                        # dnw + zero V rows instead of masks.)
                        nc.gpsimd.tensor_mul(probs[:, 4, :], probs[:, 4, :],
                                             tri[:, 1, :])
                        nc.gpsimd.tensor_mul(probs[:, 0, :], probs[:, 0, :],
                                             tri[:, 0, :])
                        if pending is not None:
                            emit_reduce(*pending)
                            if pending[3] == HKV - 1:
                                opq += [(4 * pending[1] + pending[2], j)
                                        for j in range(4)]
                        if opq:
                            emit_outproj_unit(*opq.pop(0))
                            if len(opq) > 4:
                                emit_outproj_unit(*opq.pop(0))
                        pending = (probs, blk, qc, kv)
            emit_reduce(*pending)
            opq += [(4 * pending[1] + pending[2], j) for j in range(4)]
            for unit in opq:
                emit_outproj_unit(*unit)
        qT_cm.__exit__(None, None, None)
        v_cm.__exit__(None, None, None)
        kT_cm.__exit__(None, None, None)

        probs_cm.__exit__(None, None, None)
        msk_cm.__exit__(None, None, None)
        emb_cm.__exit__(None, None, None)
        wo_cm.__exit__(None, None, None)
        acat_cm.__exit__(None, None, None)


_CACHED_NC = None


def build_graph():
    global _CACHED_NC
    if _CACHED_NC is not None:
        return _CACHED_NC
    nc = bacc.Bacc("TRN2", target_bir_lowering=False, debug=False,
                   enable_asserts=False, num_devices=8)
    d = {}
    d["emb8"] = nc.dram_tensor("emb8", [128, 3, NE, 512], f8,
                               kind="ExternalInput").ap()
    d["ident"] = nc.dram_tensor("ident", [128, 128], bf,
                                kind="ExternalInput").ap()
    d["emb_own"] = nc.dram_tensor("emb_own", [TOWN, E], bf,
                                  kind="ExternalInput").ap()
    d["wq8"] = nc.dram_tensor("wq8", [H, 128, NE, DK], f8,
                              kind="ExternalInput").ap()
    d["wk8"] = nc.dram_tensor("wk8", [128, NE, HKV * DK], f8,
                              kind="ExternalInput").ap()
    d["wv8"] = nc.dram_tensor("wv8", [128, NE, HKV * DV], f8,
                              kind="ExternalInput").ap()
    d["wo8"] = nc.dram_tensor("wo8", [128, 4, H, 512], f8,
                              kind="ExternalInput").ap()
    d["cosqT"] = nc.dram_tensor("cosqT", [DK, TOWN], bf, kind="ExternalInput").ap()
    d["sinqT"] = nc.dram_tensor("sinqT", [DK, TOWN], bf, kind="ExternalInput").ap()
    d["coskT"] = nc.dram_tensor("coskT", [DK, TALL], bf, kind="ExternalInput").ap()
    d["sinkT"] = nc.dram_tensor("sinkT", [DK, TALL], bf, kind="ExternalInput").ap()
    d["tri"] = nc.dram_tensor("tri", [128, 2, GQ * 128], f8,
                              kind="ExternalInput").ap()
    d["dnw"] = nc.dram_tensor("dnw", [128, 12, 128], f8,
                              kind="ExternalInput").ap()
    d["out"] = nc.dram_tensor("out", [TOWN, E], bf, kind="ExternalOutput").ap()

    with tile.TileContext(nc, trace_sim=False) as tc:
        build(tc, d)
    nc.compile()
    _CACHED_NC = nc
    return nc


def make_in_maps(embeddings, cos_buffer, sin_buffer, wq, wk, wv, wo):
    embeddings = np.asarray(embeddings, dtype=np.float32)
    cos_buffer = np.asarray(cos_buffer, dtype=np.float32)
    sin_buffer = np.asarray(sin_buffer, dtype=np.float32)
    # [E, H*DK] -> [H, 128, NE, DK] fp8 (k-tile-pair packed, per head).
    # Weights pre-scaled by WSCALE for e4m3 range; the whole 1/sqrt(DK)
    # score scale plus both WSCALE compensations ride on the q-side
    # cos/sin (q) and k-side cos/sin (k) host buffers.
    ws = float(WSCALE)
    wq_s = np.asarray(wq, np.float32) * ws
    wq_s = wq_s.reshape(NE, 128, H, DK).transpose(2, 1, 0, 3)
    wq8 = np.ascontiguousarray(wq_s).astype(E4M3)
    # [E, HKV*DK] -> [128, NE, HKV*DK]
    wk8 = np.ascontiguousarray(
        (np.asarray(wk, np.float32) * ws).reshape(NE, 128, HKV * DK)
        .transpose(1, 0, 2)).astype(E4M3)
    wv8 = np.ascontiguousarray(
        (np.asarray(wv, np.float32) * ws).reshape(NE, 128, HKV * DV)
        .transpose(1, 0, 2)).astype(E4M3)
    # [H*DV, E] -> [128(dv), 4(j), H, 512] (j-major output column chunks)
    wo8 = np.ascontiguousarray(
        (np.asarray(wo, np.float32) * ws).reshape(H, DV, 4, 512)
        .transpose(1, 2, 0, 3)).astype(E4M3)
    aq = 1.0 / (ws * math.sqrt(DK))   # q-side compensation (+ score scale)
    ak = 1.0 / ws                     # k-side compensation

    # triangular window masks, replicated per quad head: [128(j), 2, 4(g),
    # 128(i)] -> keep j>i for the oldest chunk, j<=i for the newest.
    jj = np.arange(128)[:, None]
    ii = np.arange(128)[None, :]
    tri = np.zeros((128, 2, GQ, 128), np.float32)
    tri[:, 0, :, :] = (jj > ii)[:, None, :]
    tri[:, 1, :, :] = (jj <= ii)[:, None, :]
    tri = tri.reshape(128, 2, GQ * 128).astype(E4M3)

    in_maps = []
    for core in range(8):
        b, c = divmod(core, 4)
        tok0 = 1024 * c
        if c == 0:
            pad = np.zeros((HALO, E), np.float32)
            seg = np.concatenate([pad, embeddings[b, :TOWN]], axis=0)
            padc = np.zeros((HALO, DK), np.float32)
            ck = np.concatenate([padc, cos_buffer[1, 0, :TOWN]], axis=0)
            sk = np.concatenate([padc, sin_buffer[1, 0, :TOWN]], axis=0)
        else:
            seg = embeddings[b, tok0 - HALO:tok0 + TOWN]
            ck = cos_buffer[1, 0, tok0 - HALO:tok0 + TOWN]
            sk = sin_buffer[1, 0, tok0 - HALO:tok0 + TOWN]

        # [TALL, E] -> [128, 3(tok chunk), NE, 512] fp8
        emb8 = np.ascontiguousarray(
            seg.T.reshape(NE, 128, 3, 512).transpose(1, 2, 0, 3)).astype(E4M3)

        # dn validity weights [128(k), 12(key tile), 128(m)]: zero for the
        # zero-padded halo tiles of the first sequence chunk so those keys
        # drop out of the softmax denominator; ones everywhere else.
        dnw = np.ones((128, 12, 128), np.float32)
        if c == 0:
            dnw[:, 0:4, :] = 0.0
        dnw = dnw.astype(E4M3)

        in_maps.append({
            "emb8": emb8,
            "ident": np.eye(128, dtype=np.float32).astype(BF16),
            "emb_own": np.ascontiguousarray(
                embeddings[b, tok0:tok0 + TOWN]).astype(BF16),
            "wq8": wq8, "wk8": wk8, "wv8": wv8, "wo8": wo8,
            "cosqT": np.ascontiguousarray(
                cos_buffer[0, 0, tok0:tok0 + TOWN].T * aq).astype(BF16),
            "sinqT": np.ascontiguousarray(
                sin_buffer[0, 0, tok0:tok0 + TOWN].T * aq).astype(BF16),
            "coskT": np.ascontiguousarray(ck.T * ak).astype(BF16),
            "sinkT": np.ascontiguousarray(sk.T * ak).astype(BF16),
            "tri": tri,
            "dnw": dnw,
        })
    return in_maps


def _install_ntff_hook():
    """Recreate the missing antenv.axon_hooks registry so
    run_bass_kernel_spmd(trace=True) can capture an NTFF profile."""
    import types
    if "antenv.axon_hooks" not in sys.modules:
        m = types.ModuleType("antenv.axon_hooks")
        m._hook = None
        m.set_axon_ntff_profile_hook = lambda h: setattr(m, "_hook", h)
        m.get_axon_ntff_profile_hook = lambda: m._hook
        sys.modules["antenv.axon_hooks"] = m
        try:
            import antenv
            antenv.axon_hooks = m
        except ImportError:
            pass
    try:
        from trn_agent_boot.trn_boot import _ntff_profile_via_ctypes
        hook = _ntff_profile_via_ctypes("/opt/axon/libaxon_pjrt.so")
        sys.modules["antenv.axon_hooks"].set_axon_ntff_profile_hook(hook)
    except Exception as exc:  # degrade to no tracing
        print(f"ntff hook install failed: {exc}", file=sys.stderr)


def kernel(embeddings, cos_buffer, sin_buffer, wq, wk, wv, wo, window_size,
           trace=False):
    assert int(window_size) == WIN
    if trace:
        _install_ntff_hook()
    nc = build_graph()
    in_maps = make_in_maps(embeddings, cos_buffer, sin_buffer, wq, wk, wv, wo)
    if trace:
        # warm-up executions: ramp device clocks so the traced run below
        # measures the steady-state rate
        for _ in range(2):
            bass_utils.run_bass_kernel_spmd(
                nc, in_maps, core_ids=list(range(8)), trace=False)
    res = bass_utils.run_bass_kernel_spmd(
        nc, in_maps, core_ids=list(range(8)), trace=trace)
    out = np.zeros((B, S, E), np.float32)
    for core in range(8):
        b, c = divmod(core, 4)
        out[b, 1024 * c:1024 * (c + 1)] = np.asarray(
            res.results[core]["out"]).astype(np.float32)
    if trace:
        kernel.last_exec_time_ns = res.exec_time_ns
    return out


kernel.last_exec_time_ns = None


# revision 46
# speedup vs baseline: 1.0983x; 1.0140x over previous
"""AthenaSA sliding-window attention layer on 8 TRN2 NeuronCores.

Sharding: sequence-parallel. 8 cores = 2 batches x 4 sequence chunks of 1024
tokens. Each core recomputes k/v for a 512-token halo (zero-padded for the
first chunk), so there are NO collectives — the kernel is embarrassingly
parallel and each core runs an identical Bass graph on different data.

Per-core pipeline (projections fp8-e4m3 with DoubleRow perf mode, all matmuls
at N=512 moving columns so the PE streams at 1 col/cycle with LDWEIGHTS fully
hidden; attention bf16 scores, accumulation f32):
  emb8 [128, 16, 1536] fp8 (host-pre-packed k-tile-pair layout) -> RMSNorm
  stats via fp8 ones-matmul partition reduction -> QK projections in
  transposed layout (q^T, k^T = [dk, tokens]) via fp8 DoubleRow + RoPE
  (partition-shifted reads) -> V projection in natural layout (fp8 DR) ->
  banded sliding-window attention processed per kv-head QUAD (the 4 GQA query
  heads of one kv head share scores/probs tiles at 512 columns), per-chunk
  softmax exp straight to fp8, triangular window masks applied on DVE, the
  first-block zero-halo correction folded into a per-core dn-weights tensor
  (halo V rows are zero so only the softmax denominator needs correcting) ->
  fp8 DoubleRow out-projection back to natural layout + residual.
"""
import math
import os
import sys

sys.path.insert(0, "/opt/trn_rl_repo")

import numpy as np
import ml_dtypes

import concourse.bass as bass
import concourse.bacc as bacc
import concourse.mybir as mybir
from concourse import tile
from concourse import bass_utils
from contextlib import ExitStack

BF16 = ml_dtypes.bfloat16
E4M3 = ml_dtypes.float8_e4m3

B, S, E = 2, 4096, 2048
H, HKV, DK, DV = 16, 4, 128, 128
WIN = 512
EPS = 1e-5
TOWN, TALL, HALO = 1024, 1536, 512
NE = E // 128            # 16 e-tiles
NP = NE // 2             # 8 e-tile pairs (DoubleRow)
NB = 2                   # window blocks per core
NQC = 4                  # query tiles of 128 per block
NCH = 5                  # key chunks of 128 per 640-window
GQ = H // HKV            # 4 query heads per kv head (one "quad")
NWARM = 26               # HAM warm-up dummy matmuls issued at t=0

f32 = mybir.dt.float32
bf = mybir.dt.bfloat16
f8 = mybir.dt.float8e4
AF = mybir.ActivationFunctionType
DR = mybir.MatmulPerfMode.DoubleRow
ALU = mybir.AluOpType

# fp8 weights are stored pre-scaled by WSCALE (power of 2) to sit in
# e4m3's normal range (raw values have sigma ~1/sqrt(E) = 0.022, partly
# denormal in e4m3). Compensated downstream: q/k via host cos/sin buffers,
# v via the rts copy scale, out-projection via the residual-add scale.
WSCALE = 32.0
# fp8 probs scale (applied via the exp bias): keeps exp(score) under e4m3
# max (240) while small probs stay above the denormal flush. Cancels
# exactly in the softmax ratio (numerator and denominator both carry it).
PSCALE = 1.0 / 8.0


def build(tc, d):
    nc = tc.nc

    with ExitStack() as stage_all:
        stage_all.enter_context(
            nc.allow_low_precision(reason="fp8/bf16 compute path by design"))
        const_pool = stage_all.enter_context(tc.tile_pool(name="const", bufs=1))
        ones = const_pool.tile([128, 128], bf)
        nc.gpsimd.memset(ones[:], 1.0)
        warmmv = const_pool.tile([128, 512], bf)
        nc.gpsimd.memset(warmmv[:], 0.0)
        epsb = const_pool.tile([128, 1], f32)
        nc.gpsimd.memset(epsb[:], EPS)
        epsw = const_pool.tile([128, 1], f32)
        nc.gpsimd.memset(epsw[:], EPS * WSCALE * WSCALE)
        # exp bias: probs = exp(score + ln(PSCALE)) = exp(score)*PSCALE
        lnps = const_pool.tile([128, 1], f32)
        nc.gpsimd.memset(lnps[:], math.log(PSCALE))

        # HAM warm-up: the PE clock-gate defaults to 4/8 (1.2 GHz) and only
        # releases after ~3.4us of sustained matmul activity. The first real
        # matmul can't start until the first 1MB emb8 chunk lands (~8-10us),
        # which would leave the whole first compute phase half-rate. Spin
        # dummy matmuls on memset tiles to pre-warm during the DMA window.
        with tc.tile_pool(name="warm_ps", bufs=1, space="PSUM") as warm_ps:
            wps = warm_ps.tile([128, 512], f32)
            for _ in range(NWARM):
                nc.tensor.matmul(wps[:], ones[:], warmmv[:],
                                 start=True, stop=True)

        # manually-scoped pools; LIFO open/close order
        acat_cm = tc.tile_pool(name="acat", bufs=HKV)          # ..D
        acat_pool = acat_cm.__enter__()
        wo_cm = tc.tile_pool(name="wo", bufs=1)               # ..D
        wo_pool = wo_cm.__enter__()
        emb_cm = tc.tile_pool(name="embown", bufs=1)          # ..D
        emb_pool = emb_cm.__enter__()
        msk_cm = tc.tile_pool(name="msk", bufs=1)             # ..D
        msk_pool = msk_cm.__enter__()
        # probs opened OUTSIDE the stage-B scope: its SBUF zone must not
        # reuse stage-B pool space, or iteration 0's exp inherits a release
        # dependency on the last Q-head's rope drain. (rec/outsb are first
        # touched several iterations in, when stage B has long drained.)
        probs_cm = tc.tile_pool(name="probs", bufs=3)         # ..D
        probs_pool = probs_cm.__enter__()
        kT_cm = tc.tile_pool(name="kT", bufs=HKV)             # ..C
        kT_pool = kT_cm.__enter__()
        v_cm = tc.tile_pool(name="v", bufs=1)                 # ..C
        v_pool = v_cm.__enter__()
        qT_cm = tc.tile_pool(name="qT", bufs=HKV)             # ..C
        qT_pool = qT_cm.__enter__()
        emb8_cm = tc.tile_pool(name="emb8", bufs=1)           # ..B2
        emb8_pool = emb8_cm.__enter__()
        wkv_cm = tc.tile_pool(name="wkv", bufs=1)             # ..B1
        wkv_pool = wkv_cm.__enter__()
        wq_cm = tc.tile_pool(name="wqp", bufs=5)              # ..B1
        wq_pool = wq_cm.__enter__()
        rb = const_pool.tile([128, TALL], bf)                 # 1/rms, all rows
        rts = const_pool.tile([128, 12], f32)                 # 1/rms per token-tile

        # emb8: whole residual-stream chunk in fp8, token-chunked
        # k-tile-pair layout [128, 3(chunk of 512 tok), NE, 512].
        # Split DMAs so several queues pull concurrently.
        # DMA order matters: chunk c0 feeds the first Gram/V matmuls — fan it
        # across every DMA queue; wv8 is needed ~2us in, before c1/c2.
        emb8 = emb8_pool.tile([128, 3, NE, 512], f8)
        for e in range(NE):
            nc.sync.dma_start(emb8[:, 0, e, :], d["emb8"][:, 0, e, :])
        ident = const_pool.tile([128, 128], bf)
        nc.sync.dma_start(ident[:], d["ident"][:])
        # chunk c1 before wv8: the c1 Gram matmuls come up before the first
        # V matmul needs wv8
        for half in range(8):
            nc.sync.dma_start(emb8[:, 1, 2 * half:2 * (half + 1), :],
                              d["emb8"][:, 1, 2 * half:2 * (half + 1), :])
        wv8 = wkv_pool.tile([128, NE, HKV * DV], f8)
        for qtr in range(4):
            nc.sync.dma_start(wv8[:, 4 * qtr:4 * (qtr + 1), :],
                              d["wv8"][:, 4 * qtr:4 * (qtr + 1), :])
        for qtr in range(4):
            nc.sync.dma_start(emb8[:, 2, 4 * qtr:4 * (qtr + 1), :],
                              d["emb8"][:, 2, 4 * qtr:4 * (qtr + 1), :])
        wk8 = wkv_pool.tile([128, NE, HKV * DK], f8)
        nc.sync.dma_start(wk8[:], d["wk8"][:])
        # tiny attention-mask / dn-weight tensors: land long before stage C
        tri = msk_pool.tile([128, 2, GQ * 128], f8)
        nc.sync.dma_start(tri[:], d["tri"][:])
        # per-(core, q-tile) chunk-0 masks: zero for the first sequence
        # chunk's dead-halo window so the c0->c4 probs pack stays correct
        tric0 = msk_pool.tile([128, 4, GQ * 128], f8)
        nc.sync.dma_start(tric0[:], d["tric0"][:])
        dnw = msk_pool.tile([128, 12, 128], f8)
        nc.sync.dma_start(dnw[:], d["dnw"][:])

        def load_wqh(h):
            """per-head wq tile, ring of 6: issue the DMA only after the
            previous occupant's matmuls are on record (WAR safety)."""
            wqh = wq_pool.tile([128, NE, DK], f8, name="wqh")
            for hf in range(2):
                nc.sync.dma_start(wqh[:, 8 * hf:8 * (hf + 1), :],
                                  d["wq8"][h][:, 8 * hf:8 * (hf + 1), :])
            return wqh

        def embsl(pe, off, w):
            """emb8 [128, 2(e pair), w] AP at global token offset off."""
            c, o = divmod(off, 512)
            assert o + w <= 512
            return emb8[:, c, 2 * pe:2 * pe + 2, o:o + w]

        # ---------------- Stage B: V, K^T, Q^T projections ----------------
        kT = []   # per kv head: [128(dk), TALL] bf16, rope'd
        qTq = []  # per kv head: [128(dk), 8 tiles x 4 heads x 128 q] bf16
        with ExitStack() as sb1:
            gi_pool = sb1.enter_context(tc.tile_pool(name="gi", bufs=3))
            r_pool = sb1.enter_context(tc.tile_pool(name="rms", bufs=1))
            cs_pool = sb1.enter_context(tc.tile_pool(name="cosk", bufs=1))
            tmp_pool = sb1.enter_context(tc.tile_pool(name="ropetmp", bufs=1))
            tmpq_pool = sb1.enter_context(tc.tile_pool(name="ropetmpq", bufs=2))

            cosk = cs_pool.tile([128, TALL], bf)
            sink = cs_pool.tile([128, TALL], bf)
            nc.sync.dma_start(cosk[:], d["coskT"][:, :])
            nc.sync.dma_start(sink[:], d["sinkT"][:, :])
            cosq = cs_pool.tile([128, TOWN], bf)
            sinq = cs_pool.tile([128, TOWN], bf)
            nc.sync.dma_start(cosq[:], d["cosqT"][:, :])
            nc.sync.dma_start(sinq[:], d["sinqT"][:, :])
            # deep wq prefetch: first 6 heads in flight before stage A ends
            wqh_tiles = [load_wqh(h) for h in range(5)]

            # ---- RMSNorm stats + V projection, pipelined per 512-token
            # chunk so PE work tracks the arriving emb8 DMAs. ssq per token
            # via PE Gram diagonals: G_t = emb_t^T emb_t (fp8 DR), GI_t =
            # G_t * I (DVE); rts (v-scale) via free-reduce(GI_t) and a
            # per-chunk sqrt/recip so V drains without waiting on rb.
            rts_raw = r_pool.tile([128, 12], f32)
            s_rt = r_pool.tile([128, 12], f32)
            gis = []
            v_all = v_pool.tile([128, 12, HKV * DV], f8)
            with ExitStack() as sa_ps:
                g_psum = sa_ps.enter_context(
                    tc.tile_pool(name="g_ps", bufs=2, space="PSUM"))
                ssq_psum = sa_ps.enter_context(
                    tc.tile_pool(name="ssq_ps", bufs=1, space="PSUM"))
                vps_pool = sa_ps.enter_context(
                    tc.tile_pool(name="v_ps", bufs=3, space="PSUM"))
                ssq = ssq_psum.tile([128, TALL], f32)  # 3 banks

                for c in range(3):
                    for t in range(4 * c, 4 * c + 4):
                        # pad G tiles to a full PSUM bank so accumulation
                        # groups of different t never share a bank
                        # (interleaved-start hazard)
                        g = g_psum.tile([128, 512], f32)
                        for pe in range(NP):
                            nc.tensor.matmul(
                                g[:, 0:128], embsl(pe, t * 128, 128),
                                embsl(pe, t * 128, 128),
                                start=(pe == 0), stop=(pe == NP - 1),
                                perf_mode=DR)
                        gi = gi_pool.tile([128, 128], bf)
                        nc.vector.tensor_mul(gi[:], g[:, 0:128], ident[:])
                        nc.vector.tensor_reduce(
                            rts_raw[:, t:t + 1], gi[:],
                            axis=mybir.AxisListType.X, op=ALU.add)
                        gis.append(gi)
                    # rts = 1/sqrt(ssq/E+eps)/WSCALE for this chunk's tiles:
                    # sqrt(WSCALE^2*(ssq/E + eps)) then plain reciprocal
                    csl = slice(4 * c, 4 * c + 4)
                    nc.scalar.activation(s_rt[:, csl], rts_raw[:, csl],
                                         AF.Sqrt, bias=epsw[:],
                                         scale=WSCALE * WSCALE / E)
                    nc.vector.reciprocal_approx_fast(rts[:, csl], s_rt[:, csl])
                    # V for this chunk's 4 token tiles; all 12 live in ONE
                    # fp8 tile so attention can take [128, 2(key-tile), 128]
                    # DoubleRow slices across tile pairs.
                    for t in range(4 * c, 4 * c + 4):
                        vps = vps_pool.tile([128, HKV * DV], f32)  # 1 bank
                        for pe in range(NP):
                            nc.tensor.matmul(
                                vps[:],
                                embsl(pe, t * 128, 128),
                                wv8[:, 2 * pe:2 * pe + 2, :],
                                start=(pe == 0), stop=(pe == NP - 1),
                                perf_mode=DR)
                        nc.vector.tensor_scalar_mul(v_all[:, t, :], vps[:],
                                                    rts[:, t:t + 1])

                # rb = 1/sqrt(ssq/E + eps), all rows identical, via
                # ssq row-broadcast = ones^T @ GI_t (single-instruction
                # groups into ssq regions are sequential-safe). Only the
                # k/q rope factors consume rb.
                for t in range(12):
                    nc.tensor.matmul(ssq[:, t * 128:(t + 1) * 128], ones[:],
                                     gis[t][:], start=True, stop=True)
                s_sb = r_pool.tile([128, TALL], f32)
                nc.scalar.activation(s_sb[:], ssq[:], AF.Sqrt,
                                     bias=epsb[:], scale=1.0 / E)
                nc.vector.reciprocal_approx_fast(s_sb[:], s_sb[:])
                nc.vector.tensor_copy(rb[:], s_sb[:])        # cast -> bf16

            nc.vector.tensor_mul(cosk[:], cosk[:], rb[:])
            nc.vector.tensor_mul(sink[:], sink[:], rb[:])
            nc.vector.tensor_mul(cosq[:], cosq[:], rb[:, HALO:])
            nc.vector.tensor_mul(sinq[:], sinq[:], rb[:, HALO:])

            with tc.tile_pool(name="q_ps", bufs=4, space="PSUM") as qps_pool:
                kps_cm = tc.tile_pool(name="k_ps", bufs=4, space="PSUM")
                kps_pool = kps_cm.__enter__()
                for hk in range(HKV):
                    # rope: ko = cos*kraw + sin*swap(kraw), emitted per
                    # 512-token chunk right behind the chunk's matmul chain
                    # so the drain tail after the last matmul stays short
                    # (the stage-C psum pools can't open until every rope
                    # read of this pool completes).
                    ksw = tmp_pool.tile([128, TALL], bf)
                    t1 = tmp_pool.tile([128, TALL], bf)
                    ko = kT_pool.tile([128, TALL], bf, name="ko")
                    for s3 in range(3):
                        kps = kps_pool.tile([128, 512], f32, name="kps")
                        for pe in range(NP):
                            nc.tensor.matmul(
                                kps[:],
                                wk8[:, 2 * pe:2 * pe + 2,
                                    hk * DK:(hk + 1) * DK],
                                embsl(pe, s3 * 512, 512),
                                start=(pe == 0), stop=(pe == NP - 1),
                                perf_mode=DR)
                        sl = slice(s3 * 512, (s3 + 1) * 512)
                        nc.scalar.copy(ksw[0:64, sl], kps[64:128, :])
                        nc.scalar.copy(ksw[64:128, sl], kps[0:64, :])
                        nc.vector.tensor_mul(t1[:, sl], kps[:], cosk[:, sl])
                        nc.vector.tensor_mul(ko[:, sl], ksw[:, sl],
                                             sink[:, sl])
                        nc.vector.tensor_add(ko[:, sl], ko[:, sl], t1[:, sl])
                    kT.append(ko)
                kps_cm.__exit__(None, None, None)

                # ---------------- Q^T projection ----------------
                # wq comes host-permuted per-head [H, 128, NE, DK] so a
                # head's weights DMA contiguously; 6-deep prefetch ring.
                for h in range(H):
                    kv, g = divmod(h, GQ)
                    if g == 0:
                        qquad = qT_pool.tile([128, 4 * TOWN], bf, name="qquad")
                        qTq.append(qquad)
                    wqh = wqh_tiles[h]
                    qsw = tmpq_pool.tile([128, TOWN], bf)
                    t1 = tmpq_pool.tile([128, TOWN], bf, name="t1q")
                    qo = qquad.rearrange(
                        "p (t g q) -> p t g q", g=GQ, q=128)[:, :, g, :]
                    for s2 in range(2):
                        qps = qps_pool.tile([128, 512], f32, name="qps")
                        for pe in range(NP):
                            nc.tensor.matmul(
                                qps[:],
                                wqh[:, 2 * pe:2 * pe + 2, :],
                                embsl(pe, HALO + s2 * 512, 512),
                                start=(pe == 0), stop=(pe == NP - 1),
                                perf_mode=DR)
                        # per-chunk rope (swap copies must ride on ACT: the
                        # cross-partition GpSimd copy crashes walrus)
                        sl = slice(s2 * 512, (s2 + 1) * 512)
                        nc.scalar.copy(qsw[0:64, sl], qps[64:128, :])
                        nc.scalar.copy(qsw[64:128, sl], qps[0:64, :])
                        nc.vector.tensor_mul(t1[:, sl], qps[:], cosq[:, sl])
                        qo2 = qo[:, 4 * s2:4 * (s2 + 1), :]
                        nc.vector.tensor_mul(qo2, qsw[:, sl], sinq[:, sl])
                        nc.vector.tensor_add(qo2, qo2, t1[:, sl])
                    if h + 5 < H:
                        wqh_tiles.append(load_wqh(h + 5))
        wq_cm.__exit__(None, None, None)
        wkv_cm.__exit__(None, None, None)
        emb8_cm.__exit__(None, None, None)

        # ---------------- Stage C: attention ----------------
        # One iteration = one (q-tile, kv-head quad): the 4 GQA query heads
        # of a kv head share the 512-column scores/probs tiles.
        acatq = []
        for kv in range(HKV):
            acatq.append(acat_pool.tile([128, 8 * GQ * 128], f8, name="acatq"))

        # out-projection weights land j-major so the first out-projection
        # only waits on its own 1MB slice; residual tiles stream in a ring
        wo8 = wo_pool.tile([128, 4, H, 512], f8)
        for j in range(4):
            nc.sync.dma_start(wo8[:, j, :, :], d["wo8"][:, j, :, :])
        emb_own = emb_pool.tile([128, 3, E], bf)
        for t in range(3):
            nc.sync.dma_start(emb_own[:, t, :],
                              d["emb_own"][t * 128:(t + 1) * 128, :])

        with ExitStack() as sc_stage:
            rec_pool = sc_stage.enter_context(tc.tile_pool(name="rec", bufs=4))
            out_pool = sc_stage.enter_context(tc.tile_pool(name="outsb", bufs=2))
            scps_pool = sc_stage.enter_context(
                tc.tile_pool(name="sc_ps", bufs=1, space="PSUM"))
            red_pool = sc_stage.enter_context(
                tc.tile_pool(name="red_ps", bufs=3, space="PSUM"))

            def emit_reduce_a(probs, blk, qc, kv):
                """First half of the reduce for one (tile, kv quad):
                attention-out + the dn pair that needs no pack, plus the
                c0->c4 probs pack (GpSimd). Returns (dn, otp) psum tiles."""
                t = 4 * blk + qc
                pr5 = probs[:]  # [128, 5, 512]
                # chunk order (1,2),(3,4),0: the first DR pair reads only
                # maskless mid-window chunks, so the chain starts on nothing
                # but the exp; the masked chunks gate only later matmuls
                dn = red_pool.tile([128, 512], f32, name="red")
                nc.tensor.matmul(dn[:], dnw[:, t + 1:t + 3, :],
                                 pr5[:, 1:3, :],
                                 start=True, stop=False, perf_mode=DR)
                otp = red_pool.tile([128, 512], f32, name="red")
                for ch in (1, 3):
                    nc.tensor.matmul(
                        otp[:],
                        v_all[:, t + ch:t + ch + 2, kv * DV:(kv + 1) * DV],
                        pr5[:, ch:ch + 2, :],
                        start=(ch == 1), stop=False, perf_mode=DR)
                nc.tensor.matmul(
                    otp[:], v_all[:, t, kv * DV:(kv + 1) * DV],
                    pr5[:, 0, :], start=False, stop=True)
                # pack the (disjoint-triangle, already masked) c0 probs into
                # the c4 slot: the denominator then closes with a single DR
                # pair. Runs on GpSimd after attention-out's c4 read; the
                # out-proj unit emitted between the two reduce halves keeps
                # the PE busy while it completes. Weight check: the packed
                # keys ride on dnw[t+4] == 1 (own tile); dead-halo c0 rows
                # were zeroed by the per-core tric0 mask.
                nc.gpsimd.tensor_add(pr5[:, 4, :], pr5[:, 4, :],
                                     pr5[:, 0, :])
                return dn, otp

            def emit_reduce_b(probs, blk, qc, kv, dn, otp):
                """Second half: close the denominator, normalize."""
                t = 4 * blk + qc
                pr5 = probs[:]
                nc.tensor.matmul(dn[:], dnw[:, t + 3:t + 5, :],
                                 pr5[:, 3:5, :],
                                 start=False, stop=True, perf_mode=DR)
                rec = rec_pool.tile([128, 512], f32)
                nc.vector.reciprocal_approx_fast(rec[:], dn[:])
                nc.vector.tensor_mul(acatq[kv][:, t * 512:(t + 1) * 512],
                                     otp[:], rec[:])

            outsb = {}

            def emit_outproj_unit(t, j):
                """One 512-column chunk of the out projection + residual for
                q-tile t. Units are spread one-per-iteration through the
                attention stream so the PE always has independent fill work
                while an iteration's exp/mask chain drains. The accumulator
                shares the red psum ring (same tag)."""
                if j == 0:
                    if 1 <= t <= 5:
                        # slot free after tile t-1's adds; stream tile t+2
                        nc.sync.dma_start(
                            emb_own[:, (t + 2) % 3, :],
                            d["emb_own"][(t + 2) * 128:(t + 3) * 128, :])
                    outsb[t] = out_pool.tile([128, E], bf, name="out_sb")
                out_sb = outsb[t]
                op = red_pool.tile([128, 512], f32, name="red")
                for kv in range(HKV):
                    for h2 in range(2):
                        pidx = 4 * kv + 2 * h2
                        lhs = acatq[kv].rearrange(
                            "p (t g q) -> p t g q", g=GQ, q=128)[
                                :, t, 2 * h2:2 * h2 + 2, :]
                        nc.tensor.matmul(
                            op[:], lhs, wo8[:, j, pidx:pidx + 2, :],
                            start=(kv == 0 and h2 == 0),
                            stop=(kv == HKV - 1 and h2 == 1),
                            perf_mode=DR)
                nc.vector.scalar_tensor_tensor(
                    out_sb[:, j * 512:(j + 1) * 512],
                    op[:], 1.0 / WSCALE,
                    emb_own[:, t % 3, j * 512:(j + 1) * 512],
                    ALU.mult, ALU.add)
                # per-slice output DMA overlaps the remaining matmuls
                nc.sync.dma_start(
                    d["out"][t * 128:(t + 1) * 128, j * 512:(j + 1) * 512],
                    out_sb[:, j * 512:(j + 1) * 512])
                if j == 3:
                    del outsb[t]

            # Software-pipelined by one (tile, quad) step: the PE queue
            # alternates scores_i / reduce_{i-1} / one out-proj unit, so the
            # reduce matmuls never sit behind a wait on their own
            # iteration's exp+mask chain.
            pending = None
            opq = []
            for blk in range(NB):
                for qc in range(NQC):
                    t = 4 * blk + qc              # own q-tile index
                    for kv in range(HKV):
                        scp = scps_pool.tile([128, NCH, 512], f32)  # 5 banks
                        probs = probs_pool.tile([128, NCH, 512], f8)
                        for ch in range(NCH):
                            nc.tensor.matmul(
                                scp[:, ch, :],
                                kT[kv][:, (t + ch) * 128:(t + ch + 1) * 128],
                                qTq[kv][:, t * 512:(t + 1) * 512],
                                start=True, stop=True)
                        # one whole-tile exp -> fp8. Must stay a single pass
                        # AFTER all five score matmuls: a partial exp read
                        # of the psum tile blocks every later score matmul
                        # into it (psum WAR tracking is tile-granular), which
                        # serializes scores against the scalar engine.
                        nc.scalar.activation(probs[:], scp[:], AF.Exp,
                                             bias=lnps[:], scale=1.0)
                        # triangular window masks on the boundary chunks, on
                        # GpSimd (its own engine: on DVE they head-of-line
                        # block rec/acat). c4 first — the reduce chains are
                        # ordered to need it before c0. (Uniform across
                        # tiles and cores; the zero-halo case is handled by
                        # dnw + zero V rows instead of masks.)
                        nc.gpsimd.tensor_mul(probs[:, 4, :], probs[:, 4, :],
                                             tri[:, 1, :])
                        c0m = tric0[:, t, :] if t < 4 else tri[:, 0, :]
                        nc.gpsimd.tensor_mul(probs[:, 0, :], probs[:, 0, :],
                                             c0m)
                        if pending is not None:
                            red = emit_reduce_a(*pending)
                            if pending[3] == HKV - 1:
                                opq += [(4 * pending[1] + pending[2], j)
                                        for j in range(4)]
                            if opq:
                                emit_outproj_unit(*opq.pop(0))
                                if len(opq) > 4:
                                    emit_outproj_unit(*opq.pop(0))
                            emit_reduce_b(*pending, *red)
                        elif opq:
                            emit_outproj_unit(*opq.pop(0))
                        pending = (probs, blk, qc, kv)
            red = emit_reduce_a(*pending)
            emit_reduce_b(*pending, *red)
            opq += [(4 * pending[1] + pending[2], j) for j in range(4)]
            for unit in opq:
                emit_outproj_unit(*unit)
        qT_cm.__exit__(None, None, None)
        v_cm.__exit__(None, None, None)
        kT_cm.__exit__(None, None, None)

        probs_cm.__exit__(None, None, None)
        msk_cm.__exit__(None, None, None)
        emb_cm.__exit__(None, None, None)
        wo_cm.__exit__(None, None, None)
        acat_cm.__exit__(None, None, None)


_CACHED_NC = None


def build_graph():
    global _CACHED_NC
    if _CACHED_NC is not None:
        return _CACHED_NC
    nc = bacc.Bacc("TRN2", target_bir_lowering=False, debug=False,
                   enable_asserts=False, num_devices=8)
    d = {}
    d["emb8"] = nc.dram_tensor("emb8", [128, 3, NE, 512], f8,
                               kind="ExternalInput").ap()
    d["ident"] = nc.dram_tensor("ident", [128, 128], bf,
                                kind="ExternalInput").ap()
    d["emb_own"] = nc.dram_tensor("emb_own", [TOWN, E], bf,
                                  kind="ExternalInput").ap()
    d["wq8"] = nc.dram_tensor("wq8", [H, 128, NE, DK], f8,
                              kind="ExternalInput").ap()
    d["wk8"] = nc.dram_tensor("wk8", [128, NE, HKV * DK], f8,
                              kind="ExternalInput").ap()
    d["wv8"] = nc.dram_tensor("wv8", [128, NE, HKV * DV], f8,
                              kind="ExternalInput").ap()
    d["wo8"] = nc.dram_tensor("wo8", [128, 4, H, 512], f8,
                              kind="ExternalInput").ap()
    d["cosqT"] = nc.dram_tensor("cosqT", [DK, TOWN], bf, kind="ExternalInput").ap()
    d["sinqT"] = nc.dram_tensor("sinqT", [DK, TOWN], bf, kind="ExternalInput").ap()
    d["coskT"] = nc.dram_tensor("coskT", [DK, TALL], bf, kind="ExternalInput").ap()
    d["sinkT"] = nc.dram_tensor("sinkT", [DK, TALL], bf, kind="ExternalInput").ap()
    d["tri"] = nc.dram_tensor("tri", [128, 2, GQ * 128], f8,
                              kind="ExternalInput").ap()
    d["tric0"] = nc.dram_tensor("tric0", [128, 4, GQ * 128], f8,
                                kind="ExternalInput").ap()
    d["dnw"] = nc.dram_tensor("dnw", [128, 12, 128], f8,
                              kind="ExternalInput").ap()
    d["out"] = nc.dram_tensor("out", [TOWN, E], bf, kind="ExternalOutput").ap()

    with tile.TileContext(nc, trace_sim=False) as tc:
        build(tc, d)
    nc.compile()
    _CACHED_NC = nc
    return nc


def make_in_maps(embeddings, cos_buffer, sin_buffer, wq, wk, wv, wo):
    embeddings = np.asarray(embeddings, dtype=np.float32)
    cos_buffer = np.asarray(cos_buffer, dtype=np.float32)
    sin_buffer = np.asarray(sin_buffer, dtype=np.float32)
    # [E, H*DK] -> [H, 128, NE, DK] fp8 (k-tile-pair packed, per head).
    # Weights pre-scaled by WSCALE for e4m3 range; the whole 1/sqrt(DK)
    # score scale plus both WSCALE compensations ride on the q-side
    # cos/sin (q) and k-side cos/sin (k) host buffers.
    ws = float(WSCALE)
    wq_s = np.asarray(wq, np.float32) * ws
    wq_s = wq_s.reshape(NE, 128, H, DK).transpose(2, 1, 0, 3)
    wq8 = np.ascontiguousarray(wq_s).astype(E4M3)
    # [E, HKV*DK] -> [128, NE, HKV*DK]
    wk8 = np.ascontiguousarray(
        (np.asarray(wk, np.float32) * ws).reshape(NE, 128, HKV * DK)
        .transpose(1, 0, 2)).astype(E4M3)
    wv8 = np.ascontiguousarray(
        (np.asarray(wv, np.float32) * ws).reshape(NE, 128, HKV * DV)
        .transpose(1, 0, 2)).astype(E4M3)
    # [H*DV, E] -> [128(dv), 4(j), H, 512] (j-major output column chunks)
    wo8 = np.ascontiguousarray(
        (np.asarray(wo, np.float32) * ws).reshape(H, DV, 4, 512)
        .transpose(1, 2, 0, 3)).astype(E4M3)
    aq = 1.0 / (ws * math.sqrt(DK))   # q-side compensation (+ score scale)
    ak = 1.0 / ws                     # k-side compensation

    # triangular window masks, replicated per quad head: [128(j), 2, 4(g),
    # 128(i)] -> keep j>i for the oldest chunk, j<=i for the newest.
    jj = np.arange(128)[:, None]
    ii = np.arange(128)[None, :]
    tri = np.zeros((128, 2, GQ, 128), np.float32)
    tri[:, 0, :, :] = (jj > ii)[:, None, :]
    tri[:, 1, :, :] = (jj <= ii)[:, None, :]
    tri = tri.reshape(128, 2, GQ * 128).astype(E4M3)

    in_maps = []
    for core in range(8):
        b, c = divmod(core, 4)
        tok0 = 1024 * c
        if c == 0:
            pad = np.zeros((HALO, E), np.float32)
            seg = np.concatenate([pad, embeddings[b, :TOWN]], axis=0)
            padc = np.zeros((HALO, DK), np.float32)
            ck = np.concatenate([padc, cos_buffer[1, 0, :TOWN]], axis=0)
            sk = np.concatenate([padc, sin_buffer[1, 0, :TOWN]], axis=0)
        else:
            seg = embeddings[b, tok0 - HALO:tok0 + TOWN]
            ck = cos_buffer[1, 0, tok0 - HALO:tok0 + TOWN]
            sk = sin_buffer[1, 0, tok0 - HALO:tok0 + TOWN]

        # [TALL, E] -> [128, 3(tok chunk), NE, 512] fp8
        emb8 = np.ascontiguousarray(
            seg.T.reshape(NE, 128, 3, 512).transpose(1, 2, 0, 3)).astype(E4M3)

        # dn validity weights [128(k), 12(key tile), 128(m)]: zero for the
        # zero-padded halo tiles of the first sequence chunk so those keys
        # drop out of the softmax denominator; ones everywhere else.
        dnw = np.ones((128, 12, 128), np.float32)
        if c == 0:
            dnw[:, 0:4, :] = 0.0
        dnw = dnw.astype(E4M3)
        # per-q-tile chunk-0 masks [128, 8(t), 4(g)*128]: the usual j>i
        # triangle, except zeroed outright where chunk 0 is the dead
        # zero-padded halo (first sequence chunk, t < 4) — the c0->c4
        # probs pack would otherwise count that junk in the denominator.
        tric0 = np.broadcast_to(
            tri[:, 0, :].reshape(128, 1, GQ * 128), (128, 4, GQ * 128)
        ).copy()
        if c == 0:
            tric0[:, 0:4, :] = 0.0
        tric0 = tric0.astype(E4M3)

        in_maps.append({
            "emb8": emb8,
            "ident": np.eye(128, dtype=np.float32).astype(BF16),
            "emb_own": np.ascontiguousarray(
                embeddings[b, tok0:tok0 + TOWN]).astype(BF16),
            "wq8": wq8, "wk8": wk8, "wv8": wv8, "wo8": wo8,
            "cosqT": np.ascontiguousarray(
                cos_buffer[0, 0, tok0:tok0 + TOWN].T * aq).astype(BF16),
            "sinqT": np.ascontiguousarray(
                sin_buffer[0, 0, tok0:tok0 + TOWN].T * aq).astype(BF16),
            "coskT": np.ascontiguousarray(ck.T * ak).astype(BF16),
            "sinkT": np.ascontiguousarray(sk.T * ak).astype(BF16),
            "tri": tri,
            "tric0": tric0,
            "dnw": dnw,
        })
    return in_maps


def _install_ntff_hook():
    """Recreate the missing antenv.axon_hooks registry so
    run_bass_kernel_spmd(trace=True) can capture an NTFF profile."""
    import types
    if "antenv.axon_hooks" not in sys.modules:
        m = types.ModuleType("antenv.axon_hooks")
        m._hook = None
        m.set_axon_ntff_profile_hook = lambda h: setattr(m, "_hook", h)
        m.get_axon_ntff_profile_hook = lambda: m._hook
        sys.modules["antenv.axon_hooks"] = m
        try:
            import antenv
            antenv.axon_hooks = m
        except ImportError:
            pass
    try:
        from trn_agent_boot.trn_boot import _ntff_profile_via_ctypes
        hook = _ntff_profile_via_ctypes("/opt/axon/libaxon_pjrt.so")
        sys.modules["antenv.axon_hooks"].set_axon_ntff_profile_hook(hook)
    except Exception as exc:  # degrade to no tracing
        print(f"ntff hook install failed: {exc}", file=sys.stderr)


def kernel(embeddings, cos_buffer, sin_buffer, wq, wk, wv, wo, window_size,
           trace=False):
    assert int(window_size) == WIN
    if trace:
        _install_ntff_hook()
    nc = build_graph()
    in_maps = make_in_maps(embeddings, cos_buffer, sin_buffer, wq, wk, wv, wo)
    if trace:
        # warm-up executions: ramp device clocks so the traced run below
        # measures the steady-state rate
        for _ in range(2):
            bass_utils.run_bass_kernel_spmd(
                nc, in_maps, core_ids=list(range(8)), trace=False)
    res = bass_utils.run_bass_kernel_spmd(
        nc, in_maps, core_ids=list(range(8)), trace=trace)
    out = np.zeros((B, S, E), np.float32)
    for core in range(8):
        b, c = divmod(core, 4)
        out[b, 1024 * c:1024 * (c + 1)] = np.asarray(
            res.results[core]["out"]).astype(np.float32)
    if trace:
        kernel.last_exec_time_ns = res.exec_time_ns
    return out


kernel.last_exec_time_ns = None
